# revision 39
# baseline (speedup 1.0000x reference)
"""Trainium2 Bass kernel for nn_AttnBlock (GroupNorm + single-head attention + proj + residual).

Reference computation (per batch element b, with C=256 channels, N=64*64=4096 positions):
    h   = GroupNorm32(x) * gn_scale + gn_bias
    q,k,v = split(qkv_w @ h + qkv_b)          (channel-interleaved split: rows 3c+0/1/2)
    w   = softmax_k(q^T k / sqrt(C))          [N, N]
    a   = v @ w^T                             [C, N]
    out = proj_w @ a + proj_b + x

Sharding: 8 cores = 4 batches x 2 q-halves.  Each core gets one full batch
element (needed for GroupNorm stats and full k/v), rolled so that its own
q-half occupies columns 0:2048; it computes the attention output for those
2048 query positions only.

Device algorithm (per core):
  - GroupNorm stats via bn_stats/bn_aggr + tiny indicator matmuls; GN is
    folded into the qkv weights on-chip (W' = W.T * scale_c per input
    channel, bias chains via tiny matmuls), so `h` is never materialized.
  - q/k/v projections read raw x bitcast to float32r (full-rate fp32).
  - The attention core runs in fp8e4m3 with DoubleRow matmuls (2 fp8
    weights/cell -> 256-deep contraction in one pass at 0.5 cyc/row):
      * k-hat/q-hat are written fp8 directly by the projection bias-apply
        (ACT), channel-halves in dim1 of a [128, 2, n] tile.
      * scores sT[kt] = DR(k-hat[:, :, ksl], q-hat[:, :, qsl]): one matmul
        per 128-kpos tile, output transposed (kpos on partitions).
      * exp with a fixed -2 offset (cancels in softmax) writes fp8 e-hat
        pair tiles [128, 2, 512]; one ACT op covers a k-tile PAIR by
        reading a 2-bank PSUM s-pair tile as [128, 1024].
      * av accumulates over kt-PAIRS: DR(v-hat[:, pr, :, chalf], e-pair).
      * rowsum via DR(ones[128,2,128], e-pair) -> sums replicated across
        all 128 partitions in PSUM (no partition-broadcast needed).
      * a = av * reciprocal(rowsum), written fp8; proj-out is also a DR
        matmul; bias/residual fused in the store STT.
  - Scores run one k-tile pair AHEAD of the exp stream (and ahead of the
    av batch in the PE queue) so ACT never waits on them: the attention
    phase measures ~98% tensor / ~96% scalar engine occupancy.
  - Softmax normalization is deferred; the proj/store epilogue for block
    jb is issued inside block jb+1's loop so the PE never stalls on it.
  - PE warm-up: a dummy bf16 matmul burst gated on the last x chunk fills
    the PE-dead stats/fold window so the HAM clock-gate is at 8/8 when
    the projections start; bias chains are emitted inside the projection
    stream so big matmuls don't queue behind the tiny-matmul/DVE
    ping-pong.
"""

import numpy as np

import concourse.bass as bass
import concourse.bacc as bacc
import concourse.tile as tile
from concourse import mybir
from concourse.bass_utils import run_bass_kernel_spmd

F32 = mybir.dt.float32
F32R = mybir.dt.float32r
F8 = mybir.dt.float8e4
BF16 = mybir.dt.bfloat16
AF = mybir.ActivationFunctionType
OP = mybir.AluOpType
DR = mybir.MatmulPerfMode.DoubleRow

B, C, H, W = 4, 256, 64, 64
N = H * W               # 4096 positions
NQ = N // 2             # 2048 query positions per core
GROUPS = 32
GSIZE = C // GROUPS     # 8 channels per group
EPS = 1e-6
QB = 512                # query block (one PSUM bank of fp32)
NJB = NQ // QB          # 4 query blocks
KT = N // 128           # 32 k-position tiles
NPAIR = KT // 2         # 16 k-tile pairs (DoubleRow granularity)
NCORES = 8
EXP_OFF = -2.0          # exp offset; cancels exactly in softmax
WARM_MMS = 48           # dense warm-up burst length (HAM un-throttle)


def _indicator_constants():
    p = np.arange(128)
    gind = np.zeros((2, 128, 32), np.float32)
    for t in range(2):
        gind[t, p, t * 16 + p // GSIZE] = 1.0
    gindT = np.ascontiguousarray(np.transpose(gind, (0, 2, 1)))
    gind_pmaj = np.ascontiguousarray(
        np.transpose(gind, (1, 0, 2))).reshape(128, 64) / GSIZE
    return gind_pmaj.astype(np.float32), gindT.reshape(2 * 32, 128)


def _emit(nc, tc, d):
    """Emit the per-core program. d: dict of DRAM APs."""
    x_d, wq_d, wk_d, wv_d, pt_d = d["x"], d["wqT"], d["wkT"], d["wvT"], d["pT"]
    vec_d, out_d = d["vecs"], d["out"]
    gind_d, gindT_d = d["gind"], d["gindT"]

    import contextlib
    ctx = contextlib.ExitStack()
    with ctx:
        sing = ctx.enter_context(tc.tile_pool(name="sing", bufs=1))
        stat = ctx.enter_context(tc.tile_pool(name="stat", bufs=2))

        # ---- persistent SBUF tiles -------------------------------------
        x0r = sing.tile([128, N], F32R, name="x0")
        x1r = sing.tile([128, N], F32R, name="x1")
        kh = sing.tile([128, 2, N], F8, name="kh")    # fp8 k, channel-halves
        qh = sing.tile([128, 2, NQ], F8, name="qh")
        vh = sing.tile([128, NPAIR, 2, 256], F8, name="vh")
        wq = sing.tile([128, 2, 256], F32, name="wq")   # [c_in_part, chunk, c_out]
        wk = sing.tile([128, 2, 256], F32, name="wk")
        wv = sing.tile([128, 2, 256], F32, name="wv")
        pt = sing.tile([128, 2, 256], F32, name="pt")
        wqs = sing.tile([128, 2, 256], F32R, name="wqs")  # GN-scaled, fp32r
        wks = sing.tile([128, 2, 256], F32R, name="wks")
        wvs = sing.tile([128, 2, 256], F32R, name="wvs")
        ph = sing.tile([128, 2, 256], F8, name="ph")      # fp8 proj weights
        vecs = sing.tile([128, 5, 2], F32, name="vecs")  # gn_scale, gn_bias, bq, bk, pbe
        gind = sing.tile([128, 2, 32], F32, name="gind")
        gindT0 = sing.tile([32, 128], F32, name="gindT0")
        gindT1 = sing.tile([32, 128], F32, name="gindT1")
        ones8 = sing.tile([128, 2, 128], F8, name="ones8")
        ones_f = sing.tile([128, 128], F32, name="ones_f")
        warm_w = sing.tile([128, 128], BF16, name="warm_w")
        epst = sing.tile([32, 1], F32, name="epst")
        eoff = sing.tile([128, 1], F32, name="eoff")
        escr = sing.tile([128, 1], F32, name="escr")

        scale_c = sing.tile([128, 2], F32, name="scale_c")   # per-channel GN scale
        gnb_c = sing.tile([128, 2], F32, name="gnb_c")       # per-channel GN bias
        bq_t = sing.tile([128, 2], F32, name="bq_t")         # q bias per c_out
        bk_t = sing.tile([128, 2], F32, name="bk_t")
        bv_t = sing.tile([128, 2], F32, name="bv_t")
        ob_t = sing.tile([128, 2], F32, name="ob_t")         # final output bias

        xr0, xr1 = x0r, x1r
        x0 = x0r.bitcast(F32)    # fp32 views for stats / residual reads
        x1 = x1r.bitcast(F32)

        # ---- DMAs -------------------------------------------------------
        # x halves on two queues (sync / gpsimd); NOTHING on the scalar
        # queue so ACT is free from t=0.  Weights follow x0 on sync; small
        # tensors follow x1 on gpsimd.
        nc.vector.memset(warm_w, 1.0)
        nc.vector.memset(ones8, 1.0)
        nc.vector.memset(ones_f, 1.0)
        nc.vector.memset(epst, EPS)
        nc.vector.memset(eoff, EXP_OFF)
        XCH = 1024
        for c in range(N // XCH):
            csl = slice(c * XCH, (c + 1) * XCH)
            nc.sync.dma_start(out=x0r[:, csl], in_=x_d[0:128, csl])
            nc.scalar.dma_start(out=x1r[:, csl], in_=x_d[128:256, csl])
        for wt, wd in ((wq, wq_d), (wk, wk_d)):
            nc.sync.dma_start(out=wt, in_=wd.rearrange("(j p) o -> p j o", p=128))
        for wt, wd in ((wv, wv_d), (pt, pt_d)):
            nc.scalar.dma_start(out=wt, in_=wd.rearrange("(j p) o -> p j o", p=128))
        nc.gpsimd.dma_start(out=vecs, in_=vec_d)
        nc.gpsimd.dma_start(out=gind, in_=gind_d)
        nc.gpsimd.dma_start(out=gindT0, in_=gindT_d[0:32, :])
        nc.gpsimd.dma_start(out=gindT1, in_=gindT_d[32:64, :])

        gsc = vecs[:, 0, :]
        gbi = vecs[:, 1, :]
        bqv = vecs[:, 2, :]
        bkv = vecs[:, 3, :]
        pbe = vecs[:, 4, :]

        # ---- PE warm-up -------------------------------------------------
        # Dense burst gated on the LAST x chunks: runs in the PE-dead
        # window while bn_aggr/fold chains execute on DVE, so the HAM
        # clock-gate is at 8/8 when the projection matmuls start.  Burst
        # segments are interleaved with the fold matmuls in queue order.
        with tc.tile_pool(name="ps_warm", bufs=1, space="PSUM") as ps_warm:
            wps = ps_warm.tile([128, 128], F32, name="wps", tag="warm")

            def warm_burst(n):
                for i in range(n):
                    nc.tensor.matmul(wps[0:64, 0:64], warm_w[0:64, 0:64],
                                     warm_w[0:64, 64:128], start=True, stop=True)

            # ---- phase 1: GroupNorm statistics --------------------------
            with tc.tile_pool(name="ps_small", bufs=2, space="PSUM") as ps_small:
                bstats0 = stat.tile([128, GSIZE, 6], F32, name="bstats0", tag="bstats0", bufs=1)
                bstats1 = stat.tile([128, GSIZE, 6], F32, name="bstats1", tag="bstats1", bufs=1)
                for sg in range(GSIZE):
                    nc.vector.bn_stats(out=bstats0[:, sg, :], in_=x0[:, sg * 512:(sg + 1) * 512])
                    nc.vector.bn_stats(out=bstats1[:, sg, :], in_=x1[:, sg * 512:(sg + 1) * 512])

                # burst gated on the last chunk's stats: runs in the PE-dead
                # stats/fold window (DMA done, so no bandwidth theft), so the
                # HAM clock-gate is at 8/8 when the projections start
                gate = ps_warm.tile([1, 6], F32, name="gate", tag="hb")
                nc.tensor.matmul(gate, bstats1[:, GSIZE - 1, 0:1],
                                 bstats1[:, GSIZE - 1, :], start=True, stop=True)
                warm_burst(40)
                statsin = []
                for t, bstats in enumerate((bstats0, bstats1)):
                    mv = stat.tile([128, 2], F32, name=f"mv{t}", tag="mv")
                    nc.vector.bn_aggr(out=mv, in_=bstats)
                    si = stat.tile([128, 2], F32, name=f"si{t}", tag=f"si{t}", bufs=1)
                    nc.vector.tensor_copy(out=si[:, 0:1], in_=mv[:, 0:1])
                    nc.vector.tensor_tensor(out=si[:, 1:2], in0=mv[:, 0:1], in1=mv[:, 0:1], op=OP.mult)
                    nc.vector.tensor_tensor(out=si[:, 1:2], in0=si[:, 1:2], in1=mv[:, 1:2], op=OP.add)
                    statsin.append(si)

                gsum_ps = ps_small.tile([32, 2], F32, name="gsum_ps", tag="small")
                nc.tensor.matmul(gsum_ps, gind[:, 0, :], statsin[0], start=True, stop=False)
                nc.tensor.matmul(gsum_ps, gind[:, 1, :], statsin[1], start=False, stop=True)
                warm_burst(24)

                grp = stat.tile([32, 2], F32, name="grp", bufs=1)
                nc.vector.tensor_copy(out=grp, in_=gsum_ps)
                var_g = stat.tile([32, 1], F32, name="var_g", bufs=1)
                nc.vector.scalar_tensor_tensor(out=var_g, in0=grp[:, 0:1],
                                               scalar=grp[:, 0:1], in1=grp[:, 1:2],
                                               op0=OP.mult, op1=OP.subtract)
                nc.scalar.activation(out=var_g, in_=var_g, func=AF.Sqrt, bias=epst, scale=-1.0)
                # dummy exp reading the sqrt's output: forces queue order
                # sqrt -> exp so the ACT table swaps to exp_and_others NOW
                # (ACT idle) — identity lives in that set too, so no further
                # table load before or during attention
                nc.scalar.activation(out=escr[0:32, :], in_=var_g, func=AF.Exp, bias=epst)
                nc.vector.reciprocal(out=grp[:, 1:2], in_=var_g)  # grp = (mu_g, rstd_g)

                for t, gt in enumerate((gindT0, gindT1)):
                    bc_ps = ps_small.tile([128, 2], F32, name=f"bc_ps{t}", tag="small")
                    nc.tensor.matmul(bc_ps, gt, grp, start=True, stop=True)
                    warm_burst(10)
                    nc.vector.tensor_tensor(out=scale_c[:, t:t + 1], in0=gsc[:, t:t + 1],
                                            in1=bc_ps[:, 1:2], op=OP.mult)
                    nc.vector.tensor_tensor(out=gnb_c[:, t:t + 1], in0=bc_ps[:, 0:1],
                                            in1=scale_c[:, t:t + 1], op=OP.mult)
                    nc.vector.tensor_tensor(out=gnb_c[:, t:t + 1], in0=gbi[:, t:t + 1],
                                            in1=gnb_c[:, t:t + 1], op=OP.subtract)

                # ---- phase 2: fold GN scale into qkv weights (fp32r)
                for wsrc, wdst in ((wk, wks), (wq, wqs), (wv, wvs)):
                    for cchunk in range(2):
                        nc.vector.tensor_scalar_mul(out=wdst[:, cchunk, :], in0=wsrc[:, cchunk, :],
                                                    scalar1=scale_c[:, cchunk:cchunk + 1])
                nc.gpsimd.tensor_copy(out=ph, in_=pt)  # fp8 proj weights

                def bias_chains():
                    # beta_W = W^T @ gnb (+ input bias); tiny matmuls — they
                    # are emitted INSIDE the projection stream so the big
                    # matmuls (which need only the folded weights) don't
                    # queue behind this PE<->DVE ping-pong
                    for wt, bsrc, bdst in ((wk, bkv, bk_t), (wq, bqv, bq_t), (wv, None, bv_t)):
                        for ot in range(2):
                            b_ps = ps_small.tile([128, 1], F32, name=f"b_ps{ot}", tag="small")
                            nc.tensor.matmul(b_ps, wt[:, 0, ot * 128:(ot + 1) * 128],
                                             gnb_c[:, 0:1], start=True, stop=False)
                            nc.tensor.matmul(b_ps, wt[:, 1, ot * 128:(ot + 1) * 128],
                                             gnb_c[:, 1:2], start=False, stop=True)
                            if bsrc is not None:
                                nc.vector.tensor_tensor(out=bdst[:, ot:ot + 1], in0=b_ps,
                                                        in1=bsrc[:, ot:ot + 1], op=OP.add)
                            else:
                                nc.vector.tensor_copy(out=bdst[:, ot:ot + 1], in_=b_ps)
                    for ot in range(2):
                        d_ps = ps_small.tile([128, 1], F32, name=f"d_ps{ot}", tag="small")
                        nc.tensor.matmul(d_ps, pt[:, 0, ot * 128:(ot + 1) * 128],
                                         bv_t[:, 0:1], start=True, stop=False)
                        nc.tensor.matmul(d_ps, pt[:, 1, ot * 128:(ot + 1) * 128],
                                         bv_t[:, 1:2], start=False, stop=True)
                        nc.vector.tensor_tensor(out=ob_t[:, ot:ot + 1], in0=d_ps,
                                                in1=pbe[:, ot:ot + 1], op=OP.add)

                # ---- phase 3: q / k / vT projections (fp32r in, fp8 out)
                with tc.tile_pool(name="ps_proj3", bufs=4, space="PSUM") as ps3:
                    # blocks whose fp8 outputs attention consumes EARLY get
                    # their bias-apply on ACT (they precede the exps in the
                    # ACT FIFO); late-consumed blocks apply on DVE so the
                    # exp stream starts as soon as k0-3 + q-jb0 are ready
                    big = []  # (dst, bias, weights, ot, jb, on_act)
                    for ot in range(2):
                        for jb in range(N // QB):
                            big.append((kh, bk_t[:, ot:ot + 1], wks, ot, jb, True))
                    for ot in range(2):
                        for jb in range(NJB):
                            big.append((qh, bq_t[:, ot:ot + 1], wqs, ot, jb, True))

                    def apply_bias(dst, bias, ot, sl, p_b, on_act):
                        if on_act:
                            nc.scalar.activation(out=dst[:, ot, sl], in_=p_b,
                                                 func=AF.Identity, bias=bias, scale=1.0)
                        else:
                            nc.vector.tensor_scalar_add(out=dst[:, ot, sl], in0=p_b,
                                                        scalar1=bias)

                    deferred = []
                    for nt in range(KT):
                        if big and nt % 4 != 3:   # 24 big blocks over 32 nt slots
                            dst, bias, wgt, ot, jb, on_act = big.pop(0)
                            sl = slice(jb * QB, (jb + 1) * QB)
                            p_b = ps3.tile([128, QB], F32, name="p_b", tag="pp")
                            nc.tensor.matmul(p_b, wgt[:, 0, ot * 128:(ot + 1) * 128],
                                             xr0[:, sl], start=True, stop=False)
                            nc.tensor.matmul(p_b, wgt[:, 1, ot * 128:(ot + 1) * 128],
                                             xr1[:, sl], start=False, stop=True)
                            if nt < 2:
                                # bias producers (bias_chains) are emitted at
                                # nt==1 — defer these applies until after
                                deferred.append((dst, bias, ot, sl, p_b, on_act))
                            else:
                                apply_bias(dst, bias, ot, sl, p_b, on_act)
                        nsl = slice(nt * 128, (nt + 1) * 128)
                        p_v = ps3.tile([128, 256], F32, name="p_v", tag="pp")
                        nc.tensor.matmul(p_v, xr0[:, nsl], wvs[:, 0, :], start=True, stop=False)
                        nc.tensor.matmul(p_v, xr1[:, nsl], wvs[:, 1, :], start=False, stop=True)
                        nc.vector.tensor_copy(out=vh[:, nt // 2, nt % 2, :], in_=p_v)
                        if nt == 1:
                            bias_chains()
                            for args in deferred:
                                apply_bias(*args)
                            deferred = []
                    assert not big

        # ---- phase 4: attention (fp8 DoubleRow) -------------------------
        with (
            tc.tile_pool(name="ps_s", bufs=2, space="PSUM") as ps_s,
            tc.tile_pool(name="ps_av", bufs=3, space="PSUM") as ps_av,
            tc.tile_pool(name="ps_po", bufs=1, space="PSUM") as ps_po,
            tc.tile_pool(name="e_pool", bufs=3) as e_pool,
            tc.tile_pool(name="an_pool", bufs=2) as an_pool,
            tc.tile_pool(name="o_pool", bufs=4) as o_pool,
            tc.tile_pool(name="rs_pool", bufs=2) as rs_pool,
        ):
            def epilogue(jb, an):
                # proj DR matmuls + bias/residual + store for query block jb.
                qsl = slice(jb * QB, (jb + 1) * QB)
                for ot, xres in enumerate((x0, x1)):
                    po = ps_po.tile([128, QB], F32, name="po", tag="po")
                    nc.tensor.matmul(po, ph[:, :, ot * 128:(ot + 1) * 128],
                                     an, start=True, stop=True, perf_mode=DR)
                    o_sb = o_pool.tile([128, QB], F32, name="o_sb", tag="o_sb")
                    nc.vector.scalar_tensor_tensor(out=o_sb, in0=po,
                                                   scalar=ob_t[:, ot:ot + 1],
                                                   in1=xres[:, qsl],
                                                   op0=OP.add, op1=OP.add)
                    nc.sync.dma_start(out=out_d[ot * 128:(ot + 1) * 128, qsl], in_=o_sb)

            pending = None
            for jb in range(NJB):
                qsl = slice(jb * QB, (jb + 1) * QB)
                av_a = ps_av.tile([128, QB], F32, name="av_a", tag="av")
                av_b = ps_av.tile([128, QB], F32, name="av_b", tag="av")
                rs = ps_av.tile([128, QB], F32, name="rs", tag="av")
                e8s = {}

                def av_group(pr):
                    e8 = e8s.pop(pr)
                    # rowsum: ones-weights DR matmul gives the softmax sums
                    # replicated across all 128 partitions (no broadcast op)
                    nc.tensor.matmul(rs, ones8, e8,
                                     start=(pr == 0), stop=(pr == NPAIR - 1), perf_mode=DR)
                    nc.tensor.matmul(av_a, vh[:, pr, :, 0:128], e8,
                                     start=(pr == 0), stop=(pr == NPAIR - 1), perf_mode=DR)
                    nc.tensor.matmul(av_b, vh[:, pr, :, 128:256], e8,
                                     start=(pr == 0), stop=(pr == NPAIR - 1), perf_mode=DR)

                def emit_s(pr):
                    s_ps = ps_s.tile([128, 2, QB], F32, name="s_ps", tag="s")
                    for i in (0, 1):
                        kt = 2 * pr + i
                        ksl = slice(kt * 128, (kt + 1) * 128)
                        nc.tensor.matmul(s_ps[:, i, :], kh[:, :, ksl], qh[:, :, qsl],
                                         start=True, stop=True, perf_mode=DR)
                    return s_ps

                # scores run one pair AHEAD of the exp stream (and ahead of
                # the av batch in the PE queue) so ACT never waits on them
                s_cur = emit_s(0)
                for pr in range(NPAIR):
                    e8 = e_pool.tile([128, 2, QB], F8, name="e8", tag="e8")
                    # one exp covers the whole pair (2 PSUM banks read as one AP)
                    nc.scalar.activation(out=e8, in_=s_cur, func=AF.Exp, bias=eoff)
                    e8s[pr] = e8
                    if pr + 1 < NPAIR:
                        s_cur = emit_s(pr + 1)
                    if pr >= 1:
                        av_group(pr - 1)
                    if pr == 2 and pending is not None:
                        epilogue(*pending)
                        pending = None
                av_group(NPAIR - 1)

                if jb < NJB - 1:
                    # normalize: a = av * (1/rowsum); rs already has the sums
                    # replicated across partitions (ones-weights DR matmul)
                    rsr = rs_pool.tile([128, QB], F32, name="rsr", tag="rsr")
                    nc.vector.reciprocal_approx_fast(out=rsr, in_=rs)
                    an = an_pool.tile([128, 2, QB], F8, name="an", tag="an")
                    nc.vector.tensor_tensor(out=an[:, 0, :], in0=av_a, in1=rsr, op=OP.mult)
                    nc.vector.tensor_tensor(out=an[:, 1, :], in0=av_b, in1=rsr, op=OP.mult)
                    pending = (jb, an)
                else:
                    # final block: pipeline the normalize/proj/store chain in
                    # two half-width pieces (DVE of half 2 overlaps PE of half 1)
                    HB = QB // 2
                    for h in range(2):
                        hsl = slice(h * HB, (h + 1) * HB)
                        qsl_h = slice(jb * QB + h * HB, jb * QB + (h + 1) * HB)
                        rsr_h = rs_pool.tile([128, HB], F32, name=f"rsrh{h}", tag=f"rsrh{h}", bufs=1)
                        nc.vector.reciprocal_approx_fast(out=rsr_h, in_=rs[:, hsl])
                        an_h = an_pool.tile([128, 2, HB], F8, name=f"an_h{h}", tag="an")
                        nc.vector.tensor_tensor(out=an_h[:, 0, :], in0=av_a[:, hsl], in1=rsr_h, op=OP.mult)
                        nc.vector.tensor_tensor(out=an_h[:, 1, :], in0=av_b[:, hsl], in1=rsr_h, op=OP.mult)
                        for ot, xres in enumerate((x0, x1)):
                            po = ps_po.tile([128, HB], F32, name="po_h", tag="po")
                            nc.tensor.matmul(po, ph[:, :, ot * 128:(ot + 1) * 128],
                                             an_h, start=True, stop=True, perf_mode=DR)
                            o_sb = o_pool.tile([128, HB], F32, name="o_sb_h", tag="o_sb")
                            nc.vector.scalar_tensor_tensor(out=o_sb, in0=po,
                                                           scalar=ob_t[:, ot:ot + 1],
                                                           in1=xres[:, qsl_h],
                                                           op0=OP.add, op1=OP.add)
                            nc.sync.dma_start(out=out_d[ot * 128:(ot + 1) * 128, qsl_h],
                                              in_=o_sb)
            assert pending is None


_CACHED_NC = None


def _build_program():
    global _CACHED_NC
    if _CACHED_NC is not None:
        return _CACHED_NC
    nc = bacc.Bacc("TRN2", target_bir_lowering=False, debug=False,
                   num_devices=NCORES)
    d = {
        "x": nc.dram_tensor("x", [C, N], F32R, kind="ExternalInput").ap(),
        "wqT": nc.dram_tensor("wqT", [C, C], F32, kind="ExternalInput").ap(),
        "wkT": nc.dram_tensor("wkT", [C, C], F32, kind="ExternalInput").ap(),
        "wvT": nc.dram_tensor("wvT", [C, C], F32, kind="ExternalInput").ap(),
        "pT": nc.dram_tensor("pT", [C, C], F32, kind="ExternalInput").ap(),
        "vecs": nc.dram_tensor("vecs", [128, 10], F32, kind="ExternalInput").ap(),
        "gind": nc.dram_tensor("gind", [128, 64], F32, kind="ExternalInput").ap(),
        "gindT": nc.dram_tensor("gindT", [2 * 32, 128], F32, kind="ExternalInput").ap(),
        "out": nc.dram_tensor("out", [C, NQ], F32, kind="ExternalOutput").ap(),
    }
    with tile.TileContext(nc) as tc:
        _emit(nc, tc, d)
    nc.compile()
    _CACHED_NC = nc
    return nc


def _prep_host(x, gn_scale, gn_bias, qkv_w, qkv_b, proj_w, proj_b):
    """Host-side weight prep + per-core input maps."""
    f = np.float32
    x = np.asarray(x, f).reshape(B, C, N)
    qkv_w = np.asarray(qkv_w, f)
    qkv_b = np.asarray(qkv_b, f)
    proj_w = np.asarray(proj_w, f)
    proj_b = np.asarray(proj_b, f)
    # split the 1/sqrt(C) score scale evenly between q and k so both sit in a
    # good fp8e4m3 range
    half_scale = np.float32(C ** -0.25)

    Wq = qkv_w[0::3] * half_scale
    bq = qkv_b[0::3] * half_scale
    Wk = qkv_w[1::3] * half_scale
    bk = qkv_b[1::3] * half_scale
    Wv, bv = qkv_w[2::3], qkv_b[2::3]

    wqT = np.ascontiguousarray(Wq.T, f)
    wkT = np.ascontiguousarray(Wk.T, f)
    wvT = np.ascontiguousarray(Wv.T, f)
    pT = np.ascontiguousarray(proj_w.T, f)
    pbe = (proj_b + proj_w @ bv).astype(f)
    vstack = np.stack([np.asarray(gn_scale, f), np.asarray(gn_bias, f),
                       bq.astype(f), bk.astype(f), pbe], axis=0)  # [5, 256]
    vecs = np.ascontiguousarray(
        vstack.reshape(5, 2, 128).transpose(2, 0, 1).reshape(128, 10))
    gind, gindT = _indicator_constants()

    shared = {"wqT": wqT, "wkT": wkT, "wvT": wvT, "pT": pT, "vecs": vecs,
              "gind": gind, "gindT": gindT}
    in_maps = []
    for ci in range(NCORES):
        b, half = divmod(ci, 2)
        xb = x[b]
        if half == 1:
            xb = np.concatenate([xb[:, NQ:], xb[:, :NQ]], axis=1)
        in_maps.append({"x": np.ascontiguousarray(xb), **shared})
    return in_maps


def _assemble(results):
    out = np.empty((B, C, N), np.float32)
    for ci in range(NCORES):
        b, half = divmod(ci, 2)
        out[b][:, half * NQ:(half + 1) * NQ] = results[ci]["out"]
    return out.reshape(B, C, H, W)


def kernel(x, gn_scale, gn_bias, qkv_w, qkv_b, proj_w, proj_b):
    nc = _build_program()
    in_maps = _prep_host(x, gn_scale, gn_bias, qkv_w, qkv_b, proj_w, proj_b)
    res = run_bass_kernel_spmd(nc, in_maps, core_ids=list(range(NCORES)))
    return _assemble(res.results)


if __name__ == "__main__":
    rng = np.random.default_rng(0)
    inputs = {
        "x": rng.standard_normal((B, C, H, W), dtype=np.float32),
        "gn_scale": np.ones(C, np.float32),
        "gn_bias": np.zeros(C, np.float32),
        "qkv_w": rng.standard_normal((3 * C, C), dtype=np.float32) * C ** -0.5,
        "qkv_b": np.zeros(3 * C, np.float32),
        "proj_w": rng.standard_normal((C, C), dtype=np.float32) * C ** -0.5,
        "proj_b": np.zeros(C, np.float32),
    }
    out = kernel(**inputs)
    print("out", out.shape, out.dtype, float(np.abs(out).mean()))


# revision 45
# speedup vs baseline: 1.0169x; 1.0169x over previous
"""Trainium2 Bass kernel for nn_AttnBlock (GroupNorm + single-head attention + proj + residual).

Reference computation (per batch element b, with C=256 channels, N=64*64=4096 positions):
    h   = GroupNorm32(x) * gn_scale + gn_bias
    q,k,v = split(qkv_w @ h + qkv_b)          (channel-interleaved split: rows 3c+0/1/2)
    w   = softmax_k(q^T k / sqrt(C))          [N, N]
    a   = v @ w^T                             [C, N]
    out = proj_w @ a + proj_b + x

Sharding: 8 cores = 4 batches x 2 q-halves.  Each core gets one full batch
element (needed for GroupNorm stats and full k/v), rolled so that its own
q-half occupies columns 0:2048; it computes the attention output for those
2048 query positions only.

Device algorithm (per core):
  - GroupNorm stats via bn_stats/bn_aggr + tiny indicator matmuls; GN is
    folded into the qkv weights on-chip (W' = W.T * scale_c per input
    channel, bias chains via tiny matmuls), so `h` is never materialized.
  - q/k/v projections read raw x bitcast to float32r (full-rate fp32).
  - The attention core runs in fp8e4m3 with DoubleRow matmuls (2 fp8
    weights/cell -> 256-deep contraction in one pass at 0.5 cyc/row):
      * k-hat/q-hat are written fp8 directly by the projection bias-apply
        (ACT), channel-halves in dim1 of a [128, 2, n] tile.
      * scores sT[kt] = DR(k-hat[:, :, ksl], q-hat[:, :, qsl]): one matmul
        per 128-kpos tile, output transposed (kpos on partitions).
      * exp with a fixed -2 offset (cancels in softmax) writes fp8 e-hat
        pair tiles [128, 2, 512]; one ACT op covers a k-tile PAIR by
        reading a 2-bank PSUM s-pair tile as [128, 1024].
      * av accumulates over kt-PAIRS: DR(v-hat[:, pr, :, chalf], e-pair).
      * rowsum via DR(ones[128,2,128], e-pair) -> sums replicated across
        all 128 partitions in PSUM (no partition-broadcast needed).
      * a = av * reciprocal(rowsum), written fp8; proj-out is also a DR
        matmul; bias/residual fused in the store STT.
  - Scores run one k-tile pair AHEAD of the exp stream (and ahead of the
    av batch in the PE queue) so ACT never waits on them: the attention
    phase measures ~98% tensor / ~96% scalar engine occupancy.
  - Softmax normalization is deferred; the proj/store epilogue for block
    jb is issued inside block jb+1's loop so the PE never stalls on it.
  - PE warm-up: a dummy bf16 matmul burst gated on the last x chunk fills
    the PE-dead stats/fold window so the HAM clock-gate is at 8/8 when
    the projections start; bias chains are emitted inside the projection
    stream so big matmuls don't queue behind the tiny-matmul/DVE
    ping-pong.
"""

import numpy as np

import concourse.bass as bass
import concourse.bacc as bacc
import concourse.tile as tile
from concourse import mybir
from concourse.bass_utils import run_bass_kernel_spmd

F32 = mybir.dt.float32
F32R = mybir.dt.float32r
F8 = mybir.dt.float8e4
BF16 = mybir.dt.bfloat16
AF = mybir.ActivationFunctionType
OP = mybir.AluOpType
DR = mybir.MatmulPerfMode.DoubleRow

B, C, H, W = 4, 256, 64, 64
N = H * W               # 4096 positions
NQ = N // 2             # 2048 query positions per core
GROUPS = 32
GSIZE = C // GROUPS     # 8 channels per group
EPS = 1e-6
QB = 512                # query block (one PSUM bank of fp32)
NJB = NQ // QB          # 4 query blocks
KT = N // 128           # 32 k-position tiles
NPAIR = KT // 2         # 16 k-tile pairs (DoubleRow granularity)
NCORES = 8
EXP_OFF = -2.0          # exp offset; cancels exactly in softmax
WARM_MMS = 48           # dense warm-up burst length (HAM un-throttle)


def _indicator_constants():
    p = np.arange(128)
    gind = np.zeros((2, 128, 32), np.float32)
    for t in range(2):
        gind[t, p, t * 16 + p // GSIZE] = 1.0
    gindT = np.ascontiguousarray(np.transpose(gind, (0, 2, 1)))
    gind_pmaj = np.ascontiguousarray(
        np.transpose(gind, (1, 0, 2))).reshape(128, 64) / GSIZE
    return gind_pmaj.astype(np.float32), gindT.reshape(2 * 32, 128)


def _emit(nc, tc, d):
    """Emit the per-core program. d: dict of DRAM APs."""
    x_d, wq_d, wk_d, wv_d, pt_d = d["x"], d["wqT"], d["wkT"], d["wvT"], d["pT"]
    vec_d, out_d = d["vecs"], d["out"]
    gind_d, gindT_d = d["gind"], d["gindT"]

    import contextlib
    ctx = contextlib.ExitStack()
    with ctx:
        sing = ctx.enter_context(tc.tile_pool(name="sing", bufs=1))
        stat = ctx.enter_context(tc.tile_pool(name="stat", bufs=2))

        # ---- persistent SBUF tiles -------------------------------------
        x0r = sing.tile([128, N], F32R, name="x0")
        x1r = sing.tile([128, N], F32R, name="x1")
        x8 = sing.tile([128, 2, N], F8, name="x8")    # fp8 x, channel-halves
        kh = sing.tile([128, 2, N], F8, name="kh")    # fp8 k, channel-halves
        qh = sing.tile([128, 2, NQ], F8, name="qh")
        vh = sing.tile([128, NPAIR, 2, 256], F8, name="vh")
        wq = sing.tile([128, 2, 256], F32, name="wq")   # [c_in_part, chunk, c_out]
        wk = sing.tile([128, 2, 256], F32, name="wk")
        wv = sing.tile([128, 2, 256], F32, name="wv")
        pt = sing.tile([128, 2, 256], F32, name="pt")
        wqs = sing.tile([128, 2, 256], F8, name="wqs")   # GN-scaled, x16, fp8
        wks = sing.tile([128, 2, 256], F8, name="wks")
        wvs = sing.tile([128, 2, 256], F32R, name="wvs")
        ph = sing.tile([128, 2, 256], F8, name="ph")      # fp8 proj weights
        vecs = sing.tile([128, 5, 2], F32, name="vecs")  # gn_scale, gn_bias, bq, bk, pbe
        gind = sing.tile([128, 2, 32], F32, name="gind")
        gindT0 = sing.tile([32, 128], F32, name="gindT0")
        gindT1 = sing.tile([32, 128], F32, name="gindT1")
        ones8 = sing.tile([128, 2, 128], F8, name="ones8")
        ones_f = sing.tile([128, 128], F32, name="ones_f")
        warm_w = sing.tile([128, 128], BF16, name="warm_w")
        epst = sing.tile([32, 1], F32, name="epst")
        eoff = sing.tile([128, 1], F32, name="eoff")
        escr = sing.tile([128, 1], F32, name="escr")

        scale_c = sing.tile([128, 2], F32, name="scale_c")   # per-channel GN scale
        gnb_c = sing.tile([128, 2], F32, name="gnb_c")       # per-channel GN bias
        bq_t = sing.tile([128, 2], F32, name="bq_t")         # q bias per c_out
        bk_t = sing.tile([128, 2], F32, name="bk_t")
        bv_t = sing.tile([128, 2], F32, name="bv_t")
        ob_t = sing.tile([128, 2], F32, name="ob_t")         # final output bias

        xr0, xr1 = x0r, x1r
        x0 = x0r.bitcast(F32)    # fp32 views for stats / residual reads
        x1 = x1r.bitcast(F32)

        # ---- DMAs -------------------------------------------------------
        # x halves on two queues (sync / gpsimd); NOTHING on the scalar
        # queue so ACT is free from t=0.  Weights follow x0 on sync; small
        # tensors follow x1 on gpsimd.
        nc.vector.memset(warm_w, 1.0)
        nc.vector.memset(ones8, 1.0)
        nc.vector.memset(ones_f, 1.0)
        nc.vector.memset(epst, EPS)
        nc.vector.memset(eoff, EXP_OFF)
        XCH = 1024
        for c in range(N // XCH):
            csl = slice(c * XCH, (c + 1) * XCH)
            nc.sync.dma_start(out=x0r[:, csl], in_=x_d[0:128, csl])
            nc.scalar.dma_start(out=x1r[:, csl], in_=x_d[128:256, csl])
        for wt, wd in ((wq, wq_d), (wk, wk_d)):
            nc.sync.dma_start(out=wt, in_=wd.rearrange("(j p) o -> p j o", p=128))
        for wt, wd in ((wv, wv_d), (pt, pt_d)):
            nc.scalar.dma_start(out=wt, in_=wd.rearrange("(j p) o -> p j o", p=128))
        nc.gpsimd.dma_start(out=vecs, in_=vec_d)
        nc.gpsimd.dma_start(out=gind, in_=gind_d)
        nc.gpsimd.dma_start(out=gindT0, in_=gindT_d[0:32, :])
        nc.gpsimd.dma_start(out=gindT1, in_=gindT_d[32:64, :])

        # fp8 interleaved copy of x for the DoubleRow k/q projections —
        # cast chunk-by-chunk on the otherwise-idle ACT during the DMA
        for c in range(N // XCH):
            csl = slice(c * XCH, (c + 1) * XCH)
            nc.scalar.copy(out=x8[:, 0, csl], in_=x0[:, csl])
            nc.scalar.copy(out=x8[:, 1, csl], in_=x1[:, csl])

        gsc = vecs[:, 0, :]
        gbi = vecs[:, 1, :]
        bqv = vecs[:, 2, :]
        bkv = vecs[:, 3, :]
        pbe = vecs[:, 4, :]

        # ---- PE warm-up -------------------------------------------------
        # Dense burst gated on the LAST x chunks: runs in the PE-dead
        # window while bn_aggr/fold chains execute on DVE, so the HAM
        # clock-gate is at 8/8 when the projection matmuls start.  Burst
        # segments are interleaved with the fold matmuls in queue order.
        with tc.tile_pool(name="ps_warm", bufs=1, space="PSUM") as ps_warm:
            wps = ps_warm.tile([128, 128], F32, name="wps", tag="warm")

            def warm_burst(n):
                for i in range(n):
                    nc.tensor.matmul(wps[0:64, 0:64], warm_w[0:64, 0:64],
                                     warm_w[0:64, 64:128], start=True, stop=True)

            # ---- phase 1: GroupNorm statistics --------------------------
            with tc.tile_pool(name="ps_small", bufs=2, space="PSUM") as ps_small:
                bstats0 = stat.tile([128, GSIZE, 6], F32, name="bstats0", tag="bstats0", bufs=1)
                bstats1 = stat.tile([128, GSIZE, 6], F32, name="bstats1", tag="bstats1", bufs=1)
                for sg in range(GSIZE):
                    nc.vector.bn_stats(out=bstats0[:, sg, :], in_=x0[:, sg * 512:(sg + 1) * 512])
                    nc.vector.bn_stats(out=bstats1[:, sg, :], in_=x1[:, sg * 512:(sg + 1) * 512])

                # burst gated on the last chunk's stats: runs in the PE-dead
                # stats/fold window (DMA done, so no bandwidth theft), so the
                # HAM clock-gate is at 8/8 when the projections start
                gate = ps_warm.tile([1, 6], F32, name="gate", tag="hb")
                nc.tensor.matmul(gate, bstats1[:, GSIZE - 1, 0:1],
                                 bstats1[:, GSIZE - 1, :], start=True, stop=True)
                warm_burst(40)
                statsin = []
                for t, bstats in enumerate((bstats0, bstats1)):
                    mv = stat.tile([128, 2], F32, name=f"mv{t}", tag="mv")
                    nc.vector.bn_aggr(out=mv, in_=bstats)
                    si = stat.tile([128, 2], F32, name=f"si{t}", tag=f"si{t}", bufs=1)
                    nc.vector.tensor_copy(out=si[:, 0:1], in_=mv[:, 0:1])
                    nc.vector.tensor_tensor(out=si[:, 1:2], in0=mv[:, 0:1], in1=mv[:, 0:1], op=OP.mult)
                    nc.vector.tensor_tensor(out=si[:, 1:2], in0=si[:, 1:2], in1=mv[:, 1:2], op=OP.add)
                    statsin.append(si)

                gsum_ps = ps_small.tile([32, 2], F32, name="gsum_ps", tag="small")
                nc.tensor.matmul(gsum_ps, gind[:, 0, :], statsin[0], start=True, stop=False)
                nc.tensor.matmul(gsum_ps, gind[:, 1, :], statsin[1], start=False, stop=True)
                warm_burst(24)

                grp = stat.tile([32, 2], F32, name="grp", bufs=1)
                nc.vector.tensor_copy(out=grp, in_=gsum_ps)
                var_g = stat.tile([32, 1], F32, name="var_g", bufs=1)
                nc.vector.scalar_tensor_tensor(out=var_g, in0=grp[:, 0:1],
                                               scalar=grp[:, 0:1], in1=grp[:, 1:2],
                                               op0=OP.mult, op1=OP.subtract)
                nc.scalar.activation(out=var_g, in_=var_g, func=AF.Sqrt, bias=epst, scale=-1.0)
                # dummy exp reading the sqrt's output: forces queue order
                # sqrt -> exp so the ACT table swaps to exp_and_others NOW
                # (ACT idle) — identity lives in that set too, so no further
                # table load before or during attention
                nc.scalar.activation(out=escr[0:32, :], in_=var_g, func=AF.Exp, bias=epst)
                nc.vector.reciprocal(out=grp[:, 1:2], in_=var_g)  # grp = (mu_g, rstd_g)

                for t, gt in enumerate((gindT0, gindT1)):
                    bc_ps = ps_small.tile([128, 2], F32, name=f"bc_ps{t}", tag="small")
                    nc.tensor.matmul(bc_ps, gt, grp, start=True, stop=True)
                    warm_burst(10)
                    nc.vector.tensor_tensor(out=scale_c[:, t:t + 1], in0=gsc[:, t:t + 1],
                                            in1=bc_ps[:, 1:2], op=OP.mult)
                    nc.vector.tensor_tensor(out=gnb_c[:, t:t + 1], in0=bc_ps[:, 0:1],
                                            in1=scale_c[:, t:t + 1], op=OP.mult)
                    nc.vector.tensor_tensor(out=gnb_c[:, t:t + 1], in0=gbi[:, t:t + 1],
                                            in1=gnb_c[:, t:t + 1], op=OP.subtract)

                # ---- phase 2: fold GN scale into qkv weights; k/q weights
                # go to fp8 with a x16 prescale (entries ~1/16-sigma would
                # otherwise sit in e4m3's subnormal range); the bias-apply
                # compensates with scale=1/16
                for cchunk in range(2):
                    nc.vector.tensor_scalar(out=wks[:, cchunk, :], in0=wk[:, cchunk, :],
                                            scalar1=scale_c[:, cchunk:cchunk + 1],
                                            scalar2=16.0, op0=OP.mult, op1=OP.mult)
                    nc.vector.tensor_scalar(out=wqs[:, cchunk, :], in0=wq[:, cchunk, :],
                                            scalar1=scale_c[:, cchunk:cchunk + 1],
                                            scalar2=16.0, op0=OP.mult, op1=OP.mult)
                    nc.vector.tensor_scalar_mul(out=wvs[:, cchunk, :], in0=wv[:, cchunk, :],
                                                scalar1=scale_c[:, cchunk:cchunk + 1])
                nc.gpsimd.tensor_copy(out=ph, in_=pt)  # fp8 proj weights

                def bias_chains():
                    # beta_W = W^T @ gnb (+ input bias); tiny matmuls — they
                    # are emitted INSIDE the projection stream so the big
                    # matmuls (which need only the folded weights) don't
                    # queue behind this PE<->DVE ping-pong
                    for wt, bsrc, bdst in ((wk, bkv, bk_t), (wq, bqv, bq_t), (wv, None, bv_t)):
                        for ot in range(2):
                            b_ps = ps_small.tile([128, 1], F32, name=f"b_ps{ot}", tag="small")
                            nc.tensor.matmul(b_ps, wt[:, 0, ot * 128:(ot + 1) * 128],
                                             gnb_c[:, 0:1], start=True, stop=False)
                            nc.tensor.matmul(b_ps, wt[:, 1, ot * 128:(ot + 1) * 128],
                                             gnb_c[:, 1:2], start=False, stop=True)
                            if bsrc is not None:
                                nc.vector.tensor_tensor(out=bdst[:, ot:ot + 1], in0=b_ps,
                                                        in1=bsrc[:, ot:ot + 1], op=OP.add)
                            else:
                                nc.vector.tensor_copy(out=bdst[:, ot:ot + 1], in_=b_ps)
                    for ot in range(2):
                        d_ps = ps_small.tile([128, 1], F32, name=f"d_ps{ot}", tag="small")
                        nc.tensor.matmul(d_ps, pt[:, 0, ot * 128:(ot + 1) * 128],
                                         bv_t[:, 0:1], start=True, stop=False)
                        nc.tensor.matmul(d_ps, pt[:, 1, ot * 128:(ot + 1) * 128],
                                         bv_t[:, 1:2], start=False, stop=True)
                        nc.vector.tensor_tensor(out=ob_t[:, ot:ot + 1], in0=d_ps,
                                                in1=pbe[:, ot:ot + 1], op=OP.add)

                # ---- phase 3: q / k / vT projections (fp32r in, fp8 out)
                with tc.tile_pool(name="ps_proj3", bufs=4, space="PSUM") as ps3:
                    # blocks whose fp8 outputs attention consumes EARLY get
                    # their bias-apply on ACT (they precede the exps in the
                    # ACT FIFO); late-consumed blocks apply on DVE so the
                    # exp stream starts as soon as k0-3 + q-jb0 are ready
                    big = []  # (dst, bias, weights, ot, jb, on_act)
                    for ot in range(2):
                        for jb in range(N // QB):
                            big.append((kh, bk_t[:, ot:ot + 1], wks, ot, jb, True))
                    for ot in range(2):
                        for jb in range(NJB):
                            big.append((qh, bq_t[:, ot:ot + 1], wqs, ot, jb, True))

                    def apply_bias(dst, bias, ot, sl, p_b, on_act):
                        # undo the x16 fp8 weight prescale here
                        if on_act:
                            nc.scalar.activation(out=dst[:, ot, sl], in_=p_b,
                                                 func=AF.Identity, bias=bias, scale=0.0625)
                        else:
                            nc.vector.tensor_scalar(out=dst[:, ot, sl], in0=p_b,
                                                    scalar1=0.0625, scalar2=bias,
                                                    op0=OP.mult, op1=OP.add)

                    deferred = []
                    for nt in range(KT):
                        if big and nt % 4 != 3:   # 24 big blocks over 32 nt slots
                            dst, bias, wgt, ot, jb, on_act = big.pop(0)
                            sl = slice(jb * QB, (jb + 1) * QB)
                            p_b = ps3.tile([128, QB], F32, name="p_b", tag="pp")
                            nc.tensor.matmul(p_b, wgt[:, :, ot * 128:(ot + 1) * 128],
                                             x8[:, :, sl], start=True, stop=True,
                                             perf_mode=DR)
                            if nt < 2:
                                # bias producers (bias_chains) are emitted at
                                # nt==1 — defer these applies until after
                                deferred.append((dst, bias, ot, sl, p_b, on_act))
                            else:
                                apply_bias(dst, bias, ot, sl, p_b, on_act)
                        nsl = slice(nt * 128, (nt + 1) * 128)
                        p_v = ps3.tile([128, 256], F32, name="p_v", tag="pp")
                        nc.tensor.matmul(p_v, xr0[:, nsl], wvs[:, 0, :], start=True, stop=False)
                        nc.tensor.matmul(p_v, xr1[:, nsl], wvs[:, 1, :], start=False, stop=True)
                        nc.vector.tensor_copy(out=vh[:, nt // 2, nt % 2, :], in_=p_v)
                        if nt == 1:
                            bias_chains()
                            for args in deferred:
                                apply_bias(*args)
                            deferred = []
                    assert not big

        # ---- phase 4: attention (fp8 DoubleRow) -------------------------
        with (
            tc.tile_pool(name="ps_s", bufs=2, space="PSUM") as ps_s,
            tc.tile_pool(name="ps_av", bufs=3, space="PSUM") as ps_av,
            tc.tile_pool(name="ps_po", bufs=1, space="PSUM") as ps_po,
            tc.tile_pool(name="e_pool", bufs=3) as e_pool,
            tc.tile_pool(name="an_pool", bufs=2) as an_pool,
            tc.tile_pool(name="o_pool", bufs=4) as o_pool,
            tc.tile_pool(name="rs_pool", bufs=2) as rs_pool,
        ):
            def epilogue(jb, an):
                # proj DR matmuls + bias/residual + store for query block jb.
                qsl = slice(jb * QB, (jb + 1) * QB)
                for ot, xres in enumerate((x0, x1)):
                    po = ps_po.tile([128, QB], F32, name="po", tag="po")
                    nc.tensor.matmul(po, ph[:, :, ot * 128:(ot + 1) * 128],
                                     an, start=True, stop=True, perf_mode=DR)
                    o_sb = o_pool.tile([128, QB], F32, name="o_sb", tag="o_sb")
                    nc.vector.scalar_tensor_tensor(out=o_sb, in0=po,
                                                   scalar=ob_t[:, ot:ot + 1],
                                                   in1=xres[:, qsl],
                                                   op0=OP.add, op1=OP.add)
                    nc.sync.dma_start(out=out_d[ot * 128:(ot + 1) * 128, qsl], in_=o_sb)

            pending = None
            for jb in range(NJB):
                qsl = slice(jb * QB, (jb + 1) * QB)
                av_a = ps_av.tile([128, QB], F32, name="av_a", tag="av")
                av_b = ps_av.tile([128, QB], F32, name="av_b", tag="av")
                rs = ps_av.tile([128, QB], F32, name="rs", tag="av")
                e8s = {}

                def av_group(pr):
                    e8 = e8s.pop(pr)
                    # rowsum: ones-weights DR matmul gives the softmax sums
                    # replicated across all 128 partitions (no broadcast op)
                    nc.tensor.matmul(rs, ones8, e8,
                                     start=(pr == 0), stop=(pr == NPAIR - 1), perf_mode=DR)
                    nc.tensor.matmul(av_a, vh[:, pr, :, 0:128], e8,
                                     start=(pr == 0), stop=(pr == NPAIR - 1), perf_mode=DR)
                    nc.tensor.matmul(av_b, vh[:, pr, :, 128:256], e8,
                                     start=(pr == 0), stop=(pr == NPAIR - 1), perf_mode=DR)

                def emit_s(pr):
                    s_ps = ps_s.tile([128, 2, QB], F32, name="s_ps", tag="s")
                    for i in (0, 1):
                        kt = 2 * pr + i
                        ksl = slice(kt * 128, (kt + 1) * 128)
                        nc.tensor.matmul(s_ps[:, i, :], kh[:, :, ksl], qh[:, :, qsl],
                                         start=True, stop=True, perf_mode=DR)
                    return s_ps

                # scores run one pair AHEAD of the exp stream (and ahead of
                # the av batch in the PE queue) so ACT never waits on them
                s_cur = emit_s(0)
                for pr in range(NPAIR):
                    e8 = e_pool.tile([128, 2, QB], F8, name="e8", tag="e8")
                    # one exp covers the whole pair (2 PSUM banks read as one AP)
                    nc.scalar.activation(out=e8, in_=s_cur, func=AF.Exp, bias=eoff)
                    e8s[pr] = e8
                    if pr + 1 < NPAIR:
                        s_cur = emit_s(pr + 1)
                    if pr >= 1:
                        av_group(pr - 1)
                    if pr == 2 and pending is not None:
                        epilogue(*pending)
                        pending = None
                av_group(NPAIR - 1)

                if jb < NJB - 1:
                    # normalize: a = av * (1/rowsum); rs already has the sums
                    # replicated across partitions (ones-weights DR matmul)
                    rsr = rs_pool.tile([128, QB], F32, name="rsr", tag="rsr")
                    nc.vector.reciprocal_approx_fast(out=rsr, in_=rs)
                    an = an_pool.tile([128, 2, QB], F8, name="an", tag="an")
                    nc.vector.tensor_tensor(out=an[:, 0, :], in0=av_a, in1=rsr, op=OP.mult)
                    nc.vector.tensor_tensor(out=an[:, 1, :], in0=av_b, in1=rsr, op=OP.mult)
                    pending = (jb, an)
                else:
                    # final block: pipeline the normalize/proj/store chain in
                    # two half-width pieces (DVE of half 2 overlaps PE of half 1)
                    HB = QB // 2
                    for h in range(2):
                        hsl = slice(h * HB, (h + 1) * HB)
                        qsl_h = slice(jb * QB + h * HB, jb * QB + (h + 1) * HB)
                        rsr_h = rs_pool.tile([128, HB], F32, name=f"rsrh{h}", tag=f"rsrh{h}", bufs=1)
                        nc.vector.reciprocal_approx_fast(out=rsr_h, in_=rs[:, hsl])
                        an_h = an_pool.tile([128, 2, HB], F8, name=f"an_h{h}", tag="an")
                        nc.vector.tensor_tensor(out=an_h[:, 0, :], in0=av_a[:, hsl], in1=rsr_h, op=OP.mult)
                        nc.vector.tensor_tensor(out=an_h[:, 1, :], in0=av_b[:, hsl], in1=rsr_h, op=OP.mult)
                        for ot, xres in enumerate((x0, x1)):
                            po = ps_po.tile([128, HB], F32, name="po_h", tag="po")
                            nc.tensor.matmul(po, ph[:, :, ot * 128:(ot + 1) * 128],
                                             an_h, start=True, stop=True, perf_mode=DR)
                            o_sb = o_pool.tile([128, HB], F32, name="o_sb_h", tag="o_sb")
                            nc.vector.scalar_tensor_tensor(out=o_sb, in0=po,
                                                           scalar=ob_t[:, ot:ot + 1],
                                                           in1=xres[:, qsl_h],
                                                           op0=OP.add, op1=OP.add)
                            nc.sync.dma_start(out=out_d[ot * 128:(ot + 1) * 128, qsl_h],
                                              in_=o_sb)
            assert pending is None


_CACHED_NC = None


def _build_program():
    global _CACHED_NC
    if _CACHED_NC is not None:
        return _CACHED_NC
    nc = bacc.Bacc("TRN2", target_bir_lowering=False, debug=False,
                   num_devices=NCORES)
    d = {
        "x": nc.dram_tensor("x", [C, N], F32R, kind="ExternalInput").ap(),
        "wqT": nc.dram_tensor("wqT", [C, C], F32, kind="ExternalInput").ap(),
        "wkT": nc.dram_tensor("wkT", [C, C], F32, kind="ExternalInput").ap(),
        "wvT": nc.dram_tensor("wvT", [C, C], F32, kind="ExternalInput").ap(),
        "pT": nc.dram_tensor("pT", [C, C], F32, kind="ExternalInput").ap(),
        "vecs": nc.dram_tensor("vecs", [128, 10], F32, kind="ExternalInput").ap(),
        "gind": nc.dram_tensor("gind", [128, 64], F32, kind="ExternalInput").ap(),
        "gindT": nc.dram_tensor("gindT", [2 * 32, 128], F32, kind="ExternalInput").ap(),
        "out": nc.dram_tensor("out", [C, NQ], F32, kind="ExternalOutput").ap(),
    }
    with tile.TileContext(nc) as tc:
        _emit(nc, tc, d)
    nc.compile()
    _CACHED_NC = nc
    return nc


def _prep_host(x, gn_scale, gn_bias, qkv_w, qkv_b, proj_w, proj_b):
    """Host-side weight prep + per-core input maps."""
    f = np.float32
    x = np.asarray(x, f).reshape(B, C, N)
    qkv_w = np.asarray(qkv_w, f)
    qkv_b = np.asarray(qkv_b, f)
    proj_w = np.asarray(proj_w, f)
    proj_b = np.asarray(proj_b, f)
    # split the 1/sqrt(C) score scale evenly between q and k so both sit in a
    # good fp8e4m3 range
    half_scale = np.float32(C ** -0.25)

    Wq = qkv_w[0::3] * half_scale
    bq = qkv_b[0::3] * half_scale
    Wk = qkv_w[1::3] * half_scale
    bk = qkv_b[1::3] * half_scale
    Wv, bv = qkv_w[2::3], qkv_b[2::3]

    wqT = np.ascontiguousarray(Wq.T, f)
    wkT = np.ascontiguousarray(Wk.T, f)
    wvT = np.ascontiguousarray(Wv.T, f)
    pT = np.ascontiguousarray(proj_w.T, f)
    pbe = (proj_b + proj_w @ bv).astype(f)
    vstack = np.stack([np.asarray(gn_scale, f), np.asarray(gn_bias, f),
                       bq.astype(f), bk.astype(f), pbe], axis=0)  # [5, 256]
    vecs = np.ascontiguousarray(
        vstack.reshape(5, 2, 128).transpose(2, 0, 1).reshape(128, 10))
    gind, gindT = _indicator_constants()

    shared = {"wqT": wqT, "wkT": wkT, "wvT": wvT, "pT": pT, "vecs": vecs,
              "gind": gind, "gindT": gindT}
    in_maps = []
    for ci in range(NCORES):
        b, half = divmod(ci, 2)
        xb = x[b]
        if half == 1:
            xb = np.concatenate([xb[:, NQ:], xb[:, :NQ]], axis=1)
        in_maps.append({"x": np.ascontiguousarray(xb), **shared})
    return in_maps


def _assemble(results):
    out = np.empty((B, C, N), np.float32)
    for ci in range(NCORES):
        b, half = divmod(ci, 2)
        out[b][:, half * NQ:(half + 1) * NQ] = results[ci]["out"]
    return out.reshape(B, C, H, W)


def kernel(x, gn_scale, gn_bias, qkv_w, qkv_b, proj_w, proj_b):
    nc = _build_program()
    in_maps = _prep_host(x, gn_scale, gn_bias, qkv_w, qkv_b, proj_w, proj_b)
    res = run_bass_kernel_spmd(nc, in_maps, core_ids=list(range(NCORES)))
    return _assemble(res.results)


if __name__ == "__main__":
    rng = np.random.default_rng(0)
    inputs = {
        "x": rng.standard_normal((B, C, H, W), dtype=np.float32),
        "gn_scale": np.ones(C, np.float32),
        "gn_bias": np.zeros(C, np.float32),
        "qkv_w": rng.standard_normal((3 * C, C), dtype=np.float32) * C ** -0.5,
        "qkv_b": np.zeros(3 * C, np.float32),
        "proj_w": rng.standard_normal((C, C), dtype=np.float32) * C ** -0.5,
        "proj_b": np.zeros(C, np.float32),
    }
    out = kernel(**inputs)
    print("out", out.shape, out.dtype, float(np.abs(out).mean()))


# revision 48
# speedup vs baseline: 1.0417x; 1.0244x over previous
"""Trainium2 Bass kernel for nn_AttnBlock (GroupNorm + single-head attention + proj + residual).

Reference computation (per batch element b, with C=256 channels, N=64*64=4096 positions):
    h   = GroupNorm32(x) * gn_scale + gn_bias
    q,k,v = split(qkv_w @ h + qkv_b)          (channel-interleaved split: rows 3c+0/1/2)
    w   = softmax_k(q^T k / sqrt(C))          [N, N]
    a   = v @ w^T                             [C, N]
    out = proj_w @ a + proj_b + x

Sharding: 8 cores = 4 batches x 2 q-halves.  Each core gets one full batch
element (needed for GroupNorm stats and full k/v), rolled so that its own
q-half occupies columns 0:2048; it computes the attention output for those
2048 query positions only.

Device algorithm (per core):
  - GroupNorm stats via bn_stats/bn_aggr + tiny indicator matmuls; GN is
    folded into the qkv weights on-chip (W' = W.T * scale_c per input
    channel, bias chains via tiny matmuls), so `h` is never materialized.
  - q/k/v projections read raw x bitcast to float32r (full-rate fp32).
  - The attention core runs in fp8e4m3 with DoubleRow matmuls (2 fp8
    weights/cell -> 256-deep contraction in one pass at 0.5 cyc/row):
      * k-hat/q-hat are written fp8 directly by the projection bias-apply
        (ACT), channel-halves in dim1 of a [128, 2, n] tile.
      * scores sT[kt] = DR(k-hat[:, :, ksl], q-hat[:, :, qsl]): one matmul
        per 128-kpos tile, output transposed (kpos on partitions).
      * exp with a fixed -2 offset (cancels in softmax) writes fp8 e-hat
        pair tiles [128, 2, 512]; one ACT op covers a k-tile PAIR by
        reading a 2-bank PSUM s-pair tile as [128, 1024].
      * av accumulates over kt-PAIRS: DR(v-hat[:, pr, :, chalf], e-pair).
      * rowsum via DR(ones[128,2,128], e-pair) -> sums replicated across
        all 128 partitions in PSUM (no partition-broadcast needed).
      * a = av * reciprocal(rowsum), written fp8; proj-out is also a DR
        matmul; bias/residual fused in the store STT.
  - Scores run one k-tile pair AHEAD of the exp stream (and ahead of the
    av batch in the PE queue) so ACT never waits on them: the attention
    phase measures ~98% tensor / ~96% scalar engine occupancy.
  - Softmax normalization is deferred; the proj/store epilogue for block
    jb is issued inside block jb+1's loop so the PE never stalls on it.
  - PE warm-up: a dummy bf16 matmul burst gated on the last x chunk fills
    the PE-dead stats/fold window so the HAM clock-gate is at 8/8 when
    the projections start; bias chains are emitted inside the projection
    stream so big matmuls don't queue behind the tiny-matmul/DVE
    ping-pong.
"""

import numpy as np

import concourse.bass as bass
import concourse.bacc as bacc
import concourse.tile as tile
from concourse import mybir
from concourse.bass_utils import run_bass_kernel_spmd

F32 = mybir.dt.float32
F32R = mybir.dt.float32r
F8 = mybir.dt.float8e4
BF16 = mybir.dt.bfloat16
AF = mybir.ActivationFunctionType
OP = mybir.AluOpType
DR = mybir.MatmulPerfMode.DoubleRow

B, C, H, W = 4, 256, 64, 64
N = H * W               # 4096 positions
NQ = N // 2             # 2048 query positions per core
GROUPS = 32
GSIZE = C // GROUPS     # 8 channels per group
EPS = 1e-6
QB = 512                # query block (one PSUM bank of fp32)
NJB = NQ // QB          # 4 query blocks
KT = N // 128           # 32 k-position tiles
NPAIR = KT // 2         # 16 k-tile pairs (DoubleRow granularity)
NCORES = 8
EXP_OFF = -2.0          # exp offset; cancels exactly in softmax
WARM_MMS = 48           # dense warm-up burst length (HAM un-throttle)


def _indicator_constants():
    p = np.arange(128)
    gind = np.zeros((2, 128, 32), np.float32)
    for t in range(2):
        gind[t, p, t * 16 + p // GSIZE] = 1.0
    gindT = np.ascontiguousarray(np.transpose(gind, (0, 2, 1)))
    gind_pmaj = np.ascontiguousarray(
        np.transpose(gind, (1, 0, 2))).reshape(128, 64) / GSIZE
    return gind_pmaj.astype(np.float32), gindT.reshape(2 * 32, 128)


def _emit(nc, tc, d):
    """Emit the per-core program. d: dict of DRAM APs."""
    x_d, wq_d, wk_d, wv_d, pt_d = d["x"], d["wqT"], d["wkT"], d["wvT"], d["pT"]
    vec_d, out_d = d["vecs"], d["out"]
    gind_d, gindT_d = d["gind"], d["gindT"]

    import contextlib
    ctx = contextlib.ExitStack()
    with ctx:
        sing = ctx.enter_context(tc.tile_pool(name="sing", bufs=1))
        stat = ctx.enter_context(tc.tile_pool(name="stat", bufs=2))

        # ---- persistent SBUF tiles -------------------------------------
        x0r = sing.tile([128, N], F32R, name="x0")
        x1r = sing.tile([128, N], F32R, name="x1")
        x8 = sing.tile([128, 2, N], F8, name="x8")    # fp8 x, channel-halves
        kh = sing.tile([128, 2, N], F8, name="kh")    # fp8 k, channel-halves
        qh = sing.tile([128, 2, NQ], F8, name="qh")
        vh = sing.tile([128, NPAIR, 2, 256], F8, name="vh")
        wq = sing.tile([128, 2, 256], F32, name="wq")   # [c_in_part, chunk, c_out]
        wk = sing.tile([128, 2, 256], F32, name="wk")
        wv = sing.tile([128, 2, 256], F32, name="wv")
        pt = sing.tile([128, 2, 256], F32, name="pt")
        wqs = sing.tile([128, 2, 256], F8, name="wqs")   # GN-scaled, x16, fp8
        wks = sing.tile([128, 2, 256], F8, name="wks")
        wvs = sing.tile([128, 2, 256], F8, name="wvs")
        ph = sing.tile([128, 2, 256], F8, name="ph")      # fp8 proj weights
        vecs = sing.tile([128, 5, 2], F32, name="vecs")  # gn_scale, gn_bias, bq, bk, pbe
        gind = sing.tile([128, 2, 32], F32, name="gind")
        gindT0 = sing.tile([32, 128], F32, name="gindT0")
        gindT1 = sing.tile([32, 128], F32, name="gindT1")
        ones8 = sing.tile([128, 2, 128], F8, name="ones8")
        ones_f = sing.tile([128, 128], F32, name="ones_f")
        warm_w = sing.tile([128, 128], BF16, name="warm_w")
        epst = sing.tile([32, 1], F32, name="epst")
        eoff = sing.tile([128, 1], F32, name="eoff")
        escr = sing.tile([128, 1], F32, name="escr")

        scale_c = sing.tile([128, 2], F32, name="scale_c")   # per-channel GN scale
        gnb_c = sing.tile([128, 2], F32, name="gnb_c")       # per-channel GN bias
        bq_t = sing.tile([128, 2], F32, name="bq_t")         # q bias per c_out
        bk_t = sing.tile([128, 2], F32, name="bk_t")
        bv_t = sing.tile([128, 2], F32, name="bv_t")
        ob_t = sing.tile([128, 2], F32, name="ob_t")         # final output bias

        xr0, xr1 = x0r, x1r
        x0 = x0r.bitcast(F32)    # fp32 views for stats / residual reads
        x1 = x1r.bitcast(F32)

        # ---- DMAs -------------------------------------------------------
        # x halves on two queues (sync / gpsimd); NOTHING on the scalar
        # queue so ACT is free from t=0.  Weights follow x0 on sync; small
        # tensors follow x1 on gpsimd.
        nc.vector.memset(warm_w, 1.0)
        nc.vector.memset(ones8, 1.0)
        nc.vector.memset(ones_f, 1.0)
        nc.vector.memset(epst, EPS)
        nc.vector.memset(eoff, EXP_OFF)
        XCH = 1024
        for c in range(N // XCH):
            csl = slice(c * XCH, (c + 1) * XCH)
            nc.sync.dma_start(out=x0r[:, csl], in_=x_d[0:128, csl])
            nc.scalar.dma_start(out=x1r[:, csl], in_=x_d[128:256, csl])
        for wt, wd in ((wq, wq_d), (wk, wk_d)):
            nc.sync.dma_start(out=wt, in_=wd.rearrange("(j p) o -> p j o", p=128))
        for wt, wd in ((wv, wv_d), (pt, pt_d)):
            nc.scalar.dma_start(out=wt, in_=wd.rearrange("(j p) o -> p j o", p=128))
        nc.gpsimd.dma_start(out=vecs, in_=vec_d)
        nc.gpsimd.dma_start(out=gind, in_=gind_d)
        nc.gpsimd.dma_start(out=gindT0, in_=gindT_d[0:32, :])
        nc.gpsimd.dma_start(out=gindT1, in_=gindT_d[32:64, :])

        # fp8 interleaved copy of x for the DoubleRow k/q projections —
        # cast chunk-by-chunk on the otherwise-idle ACT during the DMA
        for c in range(N // XCH):
            csl = slice(c * XCH, (c + 1) * XCH)
            nc.scalar.copy(out=x8[:, 0, csl], in_=x0[:, csl])
            nc.scalar.copy(out=x8[:, 1, csl], in_=x1[:, csl])

        gsc = vecs[:, 0, :]
        gbi = vecs[:, 1, :]
        bqv = vecs[:, 2, :]
        bkv = vecs[:, 3, :]
        pbe = vecs[:, 4, :]

        # ---- PE warm-up -------------------------------------------------
        # Dense burst gated on the LAST x chunks: runs in the PE-dead
        # window while bn_aggr/fold chains execute on DVE, so the HAM
        # clock-gate is at 8/8 when the projection matmuls start.  Burst
        # segments are interleaved with the fold matmuls in queue order.
        with tc.tile_pool(name="ps_warm", bufs=1, space="PSUM") as ps_warm:
            wps = ps_warm.tile([128, 128], F32, name="wps", tag="warm")

            def warm_burst(n):
                for i in range(n):
                    nc.tensor.matmul(wps[0:64, 0:64], warm_w[0:64, 0:64],
                                     warm_w[0:64, 64:128], start=True, stop=True)

            # ---- phase 1: GroupNorm statistics --------------------------
            with tc.tile_pool(name="ps_small", bufs=2, space="PSUM") as ps_small:
                bstats0 = stat.tile([128, GSIZE, 6], F32, name="bstats0", tag="bstats0", bufs=1)
                bstats1 = stat.tile([128, GSIZE, 6], F32, name="bstats1", tag="bstats1", bufs=1)
                for sg in range(GSIZE):
                    nc.vector.bn_stats(out=bstats0[:, sg, :], in_=x0[:, sg * 512:(sg + 1) * 512])
                    nc.vector.bn_stats(out=bstats1[:, sg, :], in_=x1[:, sg * 512:(sg + 1) * 512])

                # burst gated on the last chunk's stats: runs in the PE-dead
                # stats/fold window (DMA done, so no bandwidth theft), so the
                # HAM clock-gate is at 8/8 when the projections start
                gate = ps_warm.tile([1, 6], F32, name="gate", tag="hb")
                nc.tensor.matmul(gate, bstats1[:, GSIZE - 1, 0:1],
                                 bstats1[:, GSIZE - 1, :], start=True, stop=True)
                warm_burst(40)
                statsin = []
                for t, bstats in enumerate((bstats0, bstats1)):
                    mv = stat.tile([128, 2], F32, name=f"mv{t}", tag="mv")
                    nc.vector.bn_aggr(out=mv, in_=bstats)
                    si = stat.tile([128, 2], F32, name=f"si{t}", tag=f"si{t}", bufs=1)
                    nc.vector.tensor_copy(out=si[:, 0:1], in_=mv[:, 0:1])
                    nc.vector.tensor_tensor(out=si[:, 1:2], in0=mv[:, 0:1], in1=mv[:, 0:1], op=OP.mult)
                    nc.vector.tensor_tensor(out=si[:, 1:2], in0=si[:, 1:2], in1=mv[:, 1:2], op=OP.add)
                    statsin.append(si)

                gsum_ps = ps_small.tile([32, 2], F32, name="gsum_ps", tag="small")
                nc.tensor.matmul(gsum_ps, gind[:, 0, :], statsin[0], start=True, stop=False)
                nc.tensor.matmul(gsum_ps, gind[:, 1, :], statsin[1], start=False, stop=True)
                warm_burst(24)

                grp = stat.tile([32, 2], F32, name="grp", bufs=1)
                nc.vector.tensor_copy(out=grp, in_=gsum_ps)
                var_g = stat.tile([32, 1], F32, name="var_g", bufs=1)
                nc.vector.scalar_tensor_tensor(out=var_g, in0=grp[:, 0:1],
                                               scalar=grp[:, 0:1], in1=grp[:, 1:2],
                                               op0=OP.mult, op1=OP.subtract)
                nc.scalar.activation(out=var_g, in_=var_g, func=AF.Sqrt, bias=epst, scale=-1.0)
                # dummy exp reading the sqrt's output: forces queue order
                # sqrt -> exp so the ACT table swaps to exp_and_others NOW
                # (ACT idle) — identity lives in that set too, so no further
                # table load before or during attention
                nc.scalar.activation(out=escr[0:32, :], in_=var_g, func=AF.Exp, bias=epst)
                nc.vector.reciprocal(out=grp[:, 1:2], in_=var_g)  # grp = (mu_g, rstd_g)

                for t, gt in enumerate((gindT0, gindT1)):
                    bc_ps = ps_small.tile([128, 2], F32, name=f"bc_ps{t}", tag="small")
                    nc.tensor.matmul(bc_ps, gt, grp, start=True, stop=True)
                    warm_burst(10)
                    nc.vector.tensor_tensor(out=scale_c[:, t:t + 1], in0=gsc[:, t:t + 1],
                                            in1=bc_ps[:, 1:2], op=OP.mult)
                    nc.vector.tensor_tensor(out=gnb_c[:, t:t + 1], in0=bc_ps[:, 0:1],
                                            in1=scale_c[:, t:t + 1], op=OP.mult)
                    nc.vector.tensor_tensor(out=gnb_c[:, t:t + 1], in0=gbi[:, t:t + 1],
                                            in1=gnb_c[:, t:t + 1], op=OP.subtract)

                # ---- phase 2: fold GN scale into qkv weights; k/q weights
                # go to fp8 with a x16 prescale (entries ~1/16-sigma would
                # otherwise sit in e4m3's subnormal range); the bias-apply
                # compensates with scale=1/16
                for cchunk in range(2):
                    nc.vector.tensor_scalar(out=wks[:, cchunk, :], in0=wk[:, cchunk, :],
                                            scalar1=scale_c[:, cchunk:cchunk + 1],
                                            scalar2=16.0, op0=OP.mult, op1=OP.mult)
                    nc.vector.tensor_scalar(out=wqs[:, cchunk, :], in0=wq[:, cchunk, :],
                                            scalar1=scale_c[:, cchunk:cchunk + 1],
                                            scalar2=16.0, op0=OP.mult, op1=OP.mult)
                    nc.vector.tensor_scalar(out=wvs[:, cchunk, :], in0=wv[:, cchunk, :],
                                            scalar1=scale_c[:, cchunk:cchunk + 1],
                                            scalar2=16.0, op0=OP.mult, op1=OP.mult)
                nc.gpsimd.tensor_copy(out=ph, in_=pt)  # fp8 proj weights

                def bias_chains():
                    # beta_W = W^T @ gnb (+ input bias); tiny matmuls — they
                    # are emitted INSIDE the projection stream so the big
                    # matmuls (which need only the folded weights) don't
                    # queue behind this PE<->DVE ping-pong
                    for wt, bsrc, bdst in ((wk, bkv, bk_t), (wq, bqv, bq_t), (wv, None, bv_t)):
                        for ot in range(2):
                            b_ps = ps_small.tile([128, 1], F32, name=f"b_ps{ot}", tag="small")
                            nc.tensor.matmul(b_ps, wt[:, 0, ot * 128:(ot + 1) * 128],
                                             gnb_c[:, 0:1], start=True, stop=False)
                            nc.tensor.matmul(b_ps, wt[:, 1, ot * 128:(ot + 1) * 128],
                                             gnb_c[:, 1:2], start=False, stop=True)
                            if bsrc is not None:
                                nc.vector.tensor_tensor(out=bdst[:, ot:ot + 1], in0=b_ps,
                                                        in1=bsrc[:, ot:ot + 1], op=OP.add)
                            else:
                                nc.vector.tensor_copy(out=bdst[:, ot:ot + 1], in_=b_ps)
                    for ot in range(2):
                        d_ps = ps_small.tile([128, 1], F32, name=f"d_ps{ot}", tag="small")
                        nc.tensor.matmul(d_ps, pt[:, 0, ot * 128:(ot + 1) * 128],
                                         bv_t[:, 0:1], start=True, stop=False)
                        nc.tensor.matmul(d_ps, pt[:, 1, ot * 128:(ot + 1) * 128],
                                         bv_t[:, 1:2], start=False, stop=True)
                        nc.vector.tensor_tensor(out=ob_t[:, ot:ot + 1], in0=d_ps,
                                                in1=pbe[:, ot:ot + 1], op=OP.add)

                # ---- phase 3: q / k / vT projections (fp32r in, fp8 out)
                with tc.tile_pool(name="ps_proj3", bufs=4, space="PSUM") as ps3:
                    # blocks whose fp8 outputs attention consumes EARLY get
                    # their bias-apply on ACT (they precede the exps in the
                    # ACT FIFO); late-consumed blocks apply on DVE so the
                    # exp stream starts as soon as k0-3 + q-jb0 are ready
                    big = []  # (dst, bias, weights, ot, jb, on_act)
                    for ot in range(2):
                        for jb in range(N // QB):
                            big.append((kh, bk_t[:, ot:ot + 1], wks, ot, jb, True))
                    for ot in range(2):
                        for jb in range(NJB):
                            big.append((qh, bq_t[:, ot:ot + 1], wqs, ot, jb, True))

                    def apply_bias(dst, bias, ot, sl, p_b, on_act):
                        # undo the x16 fp8 weight prescale here
                        if on_act:
                            nc.scalar.activation(out=dst[:, ot, sl], in_=p_b,
                                                 func=AF.Identity, bias=bias, scale=0.0625)
                        else:
                            nc.vector.tensor_scalar(out=dst[:, ot, sl], in0=p_b,
                                                    scalar1=0.0625, scalar2=bias,
                                                    op0=OP.mult, op1=OP.add)

                    deferred = []
                    for nt in range(KT):
                        if big and nt % 4 != 3:   # 24 big blocks over 32 nt slots
                            dst, bias, wgt, ot, jb, on_act = big.pop(0)
                            sl = slice(jb * QB, (jb + 1) * QB)
                            p_b = ps3.tile([128, QB], F32, name="p_b", tag="pp")
                            nc.tensor.matmul(p_b, wgt[:, :, ot * 128:(ot + 1) * 128],
                                             x8[:, :, sl], start=True, stop=True,
                                             perf_mode=DR)
                            if nt < 2:
                                # bias producers (bias_chains) are emitted at
                                # nt==1 — defer these applies until after
                                deferred.append((dst, bias, ot, sl, p_b, on_act))
                            else:
                                apply_bias(dst, bias, ot, sl, p_b, on_act)
                        nsl = slice(nt * 128, (nt + 1) * 128)
                        p_v = ps3.tile([128, 256], F32, name="p_v", tag="pp")
                        nc.tensor.matmul(p_v, x8[:, :, nsl], wvs, start=True, stop=True,
                                         perf_mode=DR)
                        nc.vector.tensor_scalar_mul(out=vh[:, nt // 2, nt % 2, :], in0=p_v,
                                                    scalar1=0.0625)
                        if nt == 1:
                            bias_chains()
                            for args in deferred:
                                apply_bias(*args)
                            deferred = []
                    assert not big

        # ---- phase 4: attention (fp8 DoubleRow) -------------------------
        with (
            tc.tile_pool(name="ps_s", bufs=2, space="PSUM") as ps_s,
            tc.tile_pool(name="ps_av", bufs=3, space="PSUM") as ps_av,
            tc.tile_pool(name="ps_po", bufs=1, space="PSUM") as ps_po,
            tc.tile_pool(name="e_pool", bufs=3) as e_pool,
            tc.tile_pool(name="an_pool", bufs=2) as an_pool,
            tc.tile_pool(name="o_pool", bufs=4) as o_pool,
            tc.tile_pool(name="rs_pool", bufs=2) as rs_pool,
        ):
            def epilogue(jb, an):
                # proj DR matmuls + bias/residual + store for query block jb.
                qsl = slice(jb * QB, (jb + 1) * QB)
                for ot, xres in enumerate((x0, x1)):
                    po = ps_po.tile([128, QB], F32, name="po", tag="po")
                    nc.tensor.matmul(po, ph[:, :, ot * 128:(ot + 1) * 128],
                                     an, start=True, stop=True, perf_mode=DR)
                    o_sb = o_pool.tile([128, QB], F32, name="o_sb", tag="o_sb")
                    nc.vector.scalar_tensor_tensor(out=o_sb, in0=po,
                                                   scalar=ob_t[:, ot:ot + 1],
                                                   in1=xres[:, qsl],
                                                   op0=OP.add, op1=OP.add)
                    nc.sync.dma_start(out=out_d[ot * 128:(ot + 1) * 128, qsl], in_=o_sb)

            pending = None
            for jb in range(NJB):
                qsl = slice(jb * QB, (jb + 1) * QB)
                av_a = ps_av.tile([128, QB], F32, name="av_a", tag="av")
                av_b = ps_av.tile([128, QB], F32, name="av_b", tag="av")
                rs = ps_av.tile([128, QB], F32, name="rs", tag="av")
                e8s = {}

                def av_group(pr):
                    e8 = e8s.pop(pr)
                    # rowsum: ones-weights DR matmul gives the softmax sums
                    # replicated across all 128 partitions (no broadcast op)
                    nc.tensor.matmul(rs, ones8, e8,
                                     start=(pr == 0), stop=(pr == NPAIR - 1), perf_mode=DR)
                    nc.tensor.matmul(av_a, vh[:, pr, :, 0:128], e8,
                                     start=(pr == 0), stop=(pr == NPAIR - 1), perf_mode=DR)
                    nc.tensor.matmul(av_b, vh[:, pr, :, 128:256], e8,
                                     start=(pr == 0), stop=(pr == NPAIR - 1), perf_mode=DR)

                def emit_s(pr):
                    s_ps = ps_s.tile([128, 2, QB], F32, name="s_ps", tag="s")
                    for i in (0, 1):
                        kt = 2 * pr + i
                        ksl = slice(kt * 128, (kt + 1) * 128)
                        nc.tensor.matmul(s_ps[:, i, :], kh[:, :, ksl], qh[:, :, qsl],
                                         start=True, stop=True, perf_mode=DR)
                    return s_ps

                # scores run one pair AHEAD of the exp stream (and ahead of
                # the av batch in the PE queue) so ACT never waits on them
                s_cur = emit_s(0)
                for pr in range(NPAIR):
                    e8 = e_pool.tile([128, 2, QB], F8, name="e8", tag="e8")
                    # one exp covers the whole pair (2 PSUM banks read as one AP)
                    nc.scalar.activation(out=e8, in_=s_cur, func=AF.Exp, bias=eoff)
                    e8s[pr] = e8
                    if pr + 1 < NPAIR:
                        s_cur = emit_s(pr + 1)
                    if pr >= 1:
                        av_group(pr - 1)
                    if pr == 2 and pending is not None:
                        epilogue(*pending)
                        pending = None
                av_group(NPAIR - 1)

                if jb < NJB - 1:
                    # normalize: a = av * (1/rowsum); rs already has the sums
                    # replicated across partitions (ones-weights DR matmul)
                    rsr = rs_pool.tile([128, QB], F32, name="rsr", tag="rsr")
                    nc.vector.reciprocal_approx_fast(out=rsr, in_=rs)
                    an = an_pool.tile([128, 2, QB], F8, name="an", tag="an")
                    nc.vector.tensor_tensor(out=an[:, 0, :], in0=av_a, in1=rsr, op=OP.mult)
                    nc.vector.tensor_tensor(out=an[:, 1, :], in0=av_b, in1=rsr, op=OP.mult)
                    pending = (jb, an)
                else:
                    # final block: pipeline the normalize/proj/store chain in
                    # two half-width pieces (DVE of half 2 overlaps PE of half 1)
                    HB = QB // 2
                    for h in range(2):
                        hsl = slice(h * HB, (h + 1) * HB)
                        qsl_h = slice(jb * QB + h * HB, jb * QB + (h + 1) * HB)
                        rsr_h = rs_pool.tile([128, HB], F32, name=f"rsrh{h}", tag=f"rsrh{h}", bufs=1)
                        nc.vector.reciprocal_approx_fast(out=rsr_h, in_=rs[:, hsl])
                        an_h = an_pool.tile([128, 2, HB], F8, name=f"an_h{h}", tag="an")
                        nc.vector.tensor_tensor(out=an_h[:, 0, :], in0=av_a[:, hsl], in1=rsr_h, op=OP.mult)
                        nc.vector.tensor_tensor(out=an_h[:, 1, :], in0=av_b[:, hsl], in1=rsr_h, op=OP.mult)
                        for ot, xres in enumerate((x0, x1)):
                            po = ps_po.tile([128, HB], F32, name="po_h", tag="po")
                            nc.tensor.matmul(po, ph[:, :, ot * 128:(ot + 1) * 128],
                                             an_h, start=True, stop=True, perf_mode=DR)
                            o_sb = o_pool.tile([128, HB], F32, name="o_sb_h", tag="o_sb")
                            nc.vector.scalar_tensor_tensor(out=o_sb, in0=po,
                                                           scalar=ob_t[:, ot:ot + 1],
                                                           in1=xres[:, qsl_h],
                                                           op0=OP.add, op1=OP.add)
                            nc.sync.dma_start(out=out_d[ot * 128:(ot + 1) * 128, qsl_h],
                                              in_=o_sb)
            assert pending is None


_CACHED_NC = None


def _build_program():
    global _CACHED_NC
    if _CACHED_NC is not None:
        return _CACHED_NC
    nc = bacc.Bacc("TRN2", target_bir_lowering=False, debug=False,
                   num_devices=NCORES)
    d = {
        "x": nc.dram_tensor("x", [C, N], F32R, kind="ExternalInput").ap(),
        "wqT": nc.dram_tensor("wqT", [C, C], F32, kind="ExternalInput").ap(),
        "wkT": nc.dram_tensor("wkT", [C, C], F32, kind="ExternalInput").ap(),
        "wvT": nc.dram_tensor("wvT", [C, C], F32, kind="ExternalInput").ap(),
        "pT": nc.dram_tensor("pT", [C, C], F32, kind="ExternalInput").ap(),
        "vecs": nc.dram_tensor("vecs", [128, 10], F32, kind="ExternalInput").ap(),
        "gind": nc.dram_tensor("gind", [128, 64], F32, kind="ExternalInput").ap(),
        "gindT": nc.dram_tensor("gindT", [2 * 32, 128], F32, kind="ExternalInput").ap(),
        "out": nc.dram_tensor("out", [C, NQ], F32, kind="ExternalOutput").ap(),
    }
    with tile.TileContext(nc) as tc:
        _emit(nc, tc, d)
    nc.compile()
    _CACHED_NC = nc
    return nc


def _prep_host(x, gn_scale, gn_bias, qkv_w, qkv_b, proj_w, proj_b):
    """Host-side weight prep + per-core input maps."""
    f = np.float32
    x = np.asarray(x, f).reshape(B, C, N)
    qkv_w = np.asarray(qkv_w, f)
    qkv_b = np.asarray(qkv_b, f)
    proj_w = np.asarray(proj_w, f)
    proj_b = np.asarray(proj_b, f)
    # split the 1/sqrt(C) score scale evenly between q and k so both sit in a
    # good fp8e4m3 range
    half_scale = np.float32(C ** -0.25)

    Wq = qkv_w[0::3] * half_scale
    bq = qkv_b[0::3] * half_scale
    Wk = qkv_w[1::3] * half_scale
    bk = qkv_b[1::3] * half_scale
    Wv, bv = qkv_w[2::3], qkv_b[2::3]

    wqT = np.ascontiguousarray(Wq.T, f)
    wkT = np.ascontiguousarray(Wk.T, f)
    wvT = np.ascontiguousarray(Wv.T, f)
    pT = np.ascontiguousarray(proj_w.T, f)
    pbe = (proj_b + proj_w @ bv).astype(f)
    vstack = np.stack([np.asarray(gn_scale, f), np.asarray(gn_bias, f),
                       bq.astype(f), bk.astype(f), pbe], axis=0)  # [5, 256]
    vecs = np.ascontiguousarray(
        vstack.reshape(5, 2, 128).transpose(2, 0, 1).reshape(128, 10))
    gind, gindT = _indicator_constants()

    shared = {"wqT": wqT, "wkT": wkT, "wvT": wvT, "pT": pT, "vecs": vecs,
              "gind": gind, "gindT": gindT}
    in_maps = []
    for ci in range(NCORES):
        b, half = divmod(ci, 2)
        xb = x[b]
        if half == 1:
            xb = np.concatenate([xb[:, NQ:], xb[:, :NQ]], axis=1)
        in_maps.append({"x": np.ascontiguousarray(xb), **shared})
    return in_maps


def _assemble(results):
    out = np.empty((B, C, N), np.float32)
    for ci in range(NCORES):
        b, half = divmod(ci, 2)
        out[b][:, half * NQ:(half + 1) * NQ] = results[ci]["out"]
    return out.reshape(B, C, H, W)


def kernel(x, gn_scale, gn_bias, qkv_w, qkv_b, proj_w, proj_b):
    nc = _build_program()
    in_maps = _prep_host(x, gn_scale, gn_bias, qkv_w, qkv_b, proj_w, proj_b)
    res = run_bass_kernel_spmd(nc, in_maps, core_ids=list(range(NCORES)))
    return _assemble(res.results)


if __name__ == "__main__":
    rng = np.random.default_rng(0)
    inputs = {
        "x": rng.standard_normal((B, C, H, W), dtype=np.float32),
        "gn_scale": np.ones(C, np.float32),
        "gn_bias": np.zeros(C, np.float32),
        "qkv_w": rng.standard_normal((3 * C, C), dtype=np.float32) * C ** -0.5,
        "qkv_b": np.zeros(3 * C, np.float32),
        "proj_w": rng.standard_normal((C, C), dtype=np.float32) * C ** -0.5,
        "proj_b": np.zeros(C, np.float32),
    }
    out = kernel(**inputs)
    print("out", out.shape, out.dtype, float(np.abs(out).mean()))


# revision 49
# speedup vs baseline: 1.0505x; 1.0084x over previous
"""Trainium2 Bass kernel for nn_AttnBlock (GroupNorm + single-head attention + proj + residual).

Reference computation (per batch element b, with C=256 channels, N=64*64=4096 positions):
    h   = GroupNorm32(x) * gn_scale + gn_bias
    q,k,v = split(qkv_w @ h + qkv_b)          (channel-interleaved split: rows 3c+0/1/2)
    w   = softmax_k(q^T k / sqrt(C))          [N, N]
    a   = v @ w^T                             [C, N]
    out = proj_w @ a + proj_b + x

Sharding: 8 cores = 4 batches x 2 q-halves.  Each core gets one full batch
element (needed for GroupNorm stats and full k/v), rolled so that its own
q-half occupies columns 0:2048; it computes the attention output for those
2048 query positions only.

Device algorithm (per core):
  - GroupNorm stats via bn_stats/bn_aggr + tiny indicator matmuls; GN is
    folded into the qkv weights on-chip (W' = W.T * scale_c per input
    channel, bias chains via tiny matmuls), so `h` is never materialized.
  - q/k/v projections are fp8 DoubleRow too: one matmul per output block
    over an fp8 interleaved copy of x (cast on the idle ACT during the x
    DMA).  The GN-folded k/q/v weights carry a x16 prescale so their
    ~1/16-sigma entries clear e4m3's subnormal floor; the bias-apply /
    vh-cast compensates with scale=1/16.
  - The attention core runs in fp8e4m3 with DoubleRow matmuls (2 fp8
    weights/cell -> 256-deep contraction in one pass at 0.5 cyc/row):
      * k-hat/q-hat are written fp8 directly by the projection bias-apply
        (ACT), channel-halves in dim1 of a [128, 2, n] tile.
      * scores sT[kt] = DR(k-hat[:, :, ksl], q-hat[:, :, qsl]): one matmul
        per 128-kpos tile, output transposed (kpos on partitions).
      * exp with a fixed -2 offset (cancels in softmax) writes fp8 e-hat
        pair tiles [128, 2, 512]; one ACT op covers a k-tile PAIR by
        reading a 2-bank PSUM s-pair tile as [128, 1024].
      * av accumulates over kt-PAIRS: DR(v-hat[:, pr, :, chalf], e-pair).
      * rowsum via DR(ones[128,2,128], e-pair) -> sums replicated across
        all 128 partitions in PSUM (no partition-broadcast needed).
      * a = av * reciprocal(rowsum), written fp8; proj-out is also a DR
        matmul; bias/residual fused in the store STT.
  - Scores run one k-tile pair AHEAD of the exp stream (and ahead of the
    av batch in the PE queue) so ACT never waits on them: the attention
    phase measures ~98% tensor / ~96% scalar engine occupancy.
  - Softmax normalization is deferred; the proj/store epilogue for block
    jb is issued inside block jb+1's loop so the PE never stalls on it.
  - PE warm-up: a dummy bf16 matmul burst gated on the last x chunk fills
    the PE-dead stats/fold window so the HAM clock-gate is at 8/8 when
    the projections start; bias chains are emitted inside the projection
    stream so big matmuls don't queue behind the tiny-matmul/DVE
    ping-pong.
"""

import numpy as np

import concourse.bass as bass
import concourse.bacc as bacc
import concourse.tile as tile
from concourse import mybir
from concourse.bass_utils import run_bass_kernel_spmd

F32 = mybir.dt.float32
F32R = mybir.dt.float32r
F8 = mybir.dt.float8e4
BF16 = mybir.dt.bfloat16
AF = mybir.ActivationFunctionType
OP = mybir.AluOpType
DR = mybir.MatmulPerfMode.DoubleRow

B, C, H, W = 4, 256, 64, 64
N = H * W               # 4096 positions
NQ = N // 2             # 2048 query positions per core
GROUPS = 32
GSIZE = C // GROUPS     # 8 channels per group
EPS = 1e-6
QB = 512                # query block (one PSUM bank of fp32)
NJB = NQ // QB          # 4 query blocks
KT = N // 128           # 32 k-position tiles
NPAIR = KT // 2         # 16 k-tile pairs (DoubleRow granularity)
NCORES = 8
EXP_OFF = -2.0          # exp offset; cancels exactly in softmax
WARM_MMS = 48           # dense warm-up burst length (HAM un-throttle)


def _indicator_constants():
    p = np.arange(128)
    gind = np.zeros((2, 128, 32), np.float32)
    for t in range(2):
        gind[t, p, t * 16 + p // GSIZE] = 1.0
    gindT = np.ascontiguousarray(np.transpose(gind, (0, 2, 1)))
    gind_pmaj = np.ascontiguousarray(
        np.transpose(gind, (1, 0, 2))).reshape(128, 64) / GSIZE
    return gind_pmaj.astype(np.float32), gindT.reshape(2 * 32, 128)


def _emit(nc, tc, d):
    """Emit the per-core program. d: dict of DRAM APs."""
    x_d, wq_d, wk_d, wv_d, pt_d = d["x"], d["wqT"], d["wkT"], d["wvT"], d["pT"]
    vec_d, out_d = d["vecs"], d["out"]
    gind_d, gindT_d = d["gind"], d["gindT"]

    import contextlib
    ctx = contextlib.ExitStack()
    with ctx:
        sing = ctx.enter_context(tc.tile_pool(name="sing", bufs=1))
        stat = ctx.enter_context(tc.tile_pool(name="stat", bufs=2))

        # ---- persistent SBUF tiles -------------------------------------
        x0r = sing.tile([128, N], F32R, name="x0")
        x1r = sing.tile([128, N], F32R, name="x1")
        x8 = sing.tile([128, 2, N], F8, name="x8")    # fp8 x, channel-halves
        kh = sing.tile([128, 2, N], F8, name="kh")    # fp8 k, channel-halves
        qh = sing.tile([128, 2, NQ], F8, name="qh")
        vh = sing.tile([128, NPAIR, 2, 256], F8, name="vh")
        wq = sing.tile([128, 2, 256], F32, name="wq")   # [c_in_part, chunk, c_out]
        wk = sing.tile([128, 2, 256], F32, name="wk")
        wv = sing.tile([128, 2, 256], F32, name="wv")
        pt = sing.tile([128, 2, 256], F32, name="pt")
        wqs = sing.tile([128, 2, 256], F8, name="wqs")   # GN-scaled, x16, fp8
        wks = sing.tile([128, 2, 256], F8, name="wks")
        wvs = sing.tile([128, 2, 256], F8, name="wvs")
        ph = sing.tile([128, 2, 256], F8, name="ph")      # fp8 proj weights
        vecs = sing.tile([128, 5, 2], F32, name="vecs")  # gn_scale, gn_bias, bq, bk, pbe
        gind = sing.tile([128, 2, 32], F32, name="gind")
        gindT0 = sing.tile([32, 128], F32, name="gindT0")
        gindT1 = sing.tile([32, 128], F32, name="gindT1")
        ones8 = sing.tile([128, 2, 128], F8, name="ones8")
        ones_f = sing.tile([128, 128], F32, name="ones_f")
        warm_w = sing.tile([128, 128], BF16, name="warm_w")
        epst = sing.tile([32, 1], F32, name="epst")
        eoff = sing.tile([128, 1], F32, name="eoff")
        escr = sing.tile([128, 1], F32, name="escr")

        scale_c = sing.tile([128, 2], F32, name="scale_c")   # per-channel GN scale
        gnb_c = sing.tile([128, 2], F32, name="gnb_c")       # per-channel GN bias
        bq_t = sing.tile([128, 2], F32, name="bq_t")         # q bias per c_out
        bk_t = sing.tile([128, 2], F32, name="bk_t")
        bv_t = sing.tile([128, 2], F32, name="bv_t")
        ob_t = sing.tile([128, 2], F32, name="ob_t")         # final output bias

        xr0, xr1 = x0r, x1r
        x0 = x0r.bitcast(F32)    # fp32 views for stats / residual reads
        x1 = x1r.bitcast(F32)

        # ---- DMAs -------------------------------------------------------
        # x halves on two queues (sync / gpsimd); NOTHING on the scalar
        # queue so ACT is free from t=0.  Weights follow x0 on sync; small
        # tensors follow x1 on gpsimd.
        nc.vector.memset(warm_w, 1.0)
        nc.vector.memset(ones8, 1.0)
        nc.vector.memset(ones_f, 1.0)
        nc.vector.memset(epst, EPS)
        nc.vector.memset(eoff, EXP_OFF)
        XCH = 1024
        for c in range(N // XCH):
            csl = slice(c * XCH, (c + 1) * XCH)
            nc.sync.dma_start(out=x0r[:, csl], in_=x_d[0:128, csl])
            nc.scalar.dma_start(out=x1r[:, csl], in_=x_d[128:256, csl])
        for wt, wd in ((wq, wq_d), (wk, wk_d)):
            nc.sync.dma_start(out=wt, in_=wd.rearrange("(j p) o -> p j o", p=128))
        for wt, wd in ((wv, wv_d), (pt, pt_d)):
            nc.scalar.dma_start(out=wt, in_=wd.rearrange("(j p) o -> p j o", p=128))
        nc.gpsimd.dma_start(out=vecs, in_=vec_d)
        nc.gpsimd.dma_start(out=gind, in_=gind_d)
        nc.gpsimd.dma_start(out=gindT0, in_=gindT_d[0:32, :])
        nc.gpsimd.dma_start(out=gindT1, in_=gindT_d[32:64, :])

        # fp8 interleaved copy of x for the DoubleRow k/q projections —
        # cast chunk-by-chunk on the otherwise-idle ACT during the DMA
        for c in range(N // XCH):
            csl = slice(c * XCH, (c + 1) * XCH)
            nc.scalar.copy(out=x8[:, 0, csl], in_=x0[:, csl])
            nc.scalar.copy(out=x8[:, 1, csl], in_=x1[:, csl])

        gsc = vecs[:, 0, :]
        gbi = vecs[:, 1, :]
        bqv = vecs[:, 2, :]
        bkv = vecs[:, 3, :]
        pbe = vecs[:, 4, :]

        # ---- PE warm-up -------------------------------------------------
        # Dense burst gated on the LAST x chunks: runs in the PE-dead
        # window while bn_aggr/fold chains execute on DVE, so the HAM
        # clock-gate is at 8/8 when the projection matmuls start.  Burst
        # segments are interleaved with the fold matmuls in queue order.
        with tc.tile_pool(name="ps_warm", bufs=1, space="PSUM") as ps_warm:
            wps = ps_warm.tile([128, 128], F32, name="wps", tag="warm")

            def warm_burst(n):
                for i in range(n):
                    nc.tensor.matmul(wps[0:64, 0:64], warm_w[0:64, 0:64],
                                     warm_w[0:64, 64:128], start=True, stop=True)

            # ---- phase 1: GroupNorm statistics --------------------------
            with tc.tile_pool(name="ps_small", bufs=2, space="PSUM") as ps_small:
                bstats0 = stat.tile([128, GSIZE, 6], F32, name="bstats0", tag="bstats0", bufs=1)
                bstats1 = stat.tile([128, GSIZE, 6], F32, name="bstats1", tag="bstats1", bufs=1)
                for sg in range(GSIZE):
                    nc.vector.bn_stats(out=bstats0[:, sg, :], in_=x0[:, sg * 512:(sg + 1) * 512])
                    nc.vector.bn_stats(out=bstats1[:, sg, :], in_=x1[:, sg * 512:(sg + 1) * 512])

                # burst gated on the last chunk's stats: runs in the PE-dead
                # stats/fold window (DMA done, so no bandwidth theft), so the
                # HAM clock-gate is at 8/8 when the projections start
                gate = ps_warm.tile([1, 6], F32, name="gate", tag="hb")
                nc.tensor.matmul(gate, bstats1[:, GSIZE - 1, 0:1],
                                 bstats1[:, GSIZE - 1, :], start=True, stop=True)
                warm_burst(40)
                statsin = []
                for t, bstats in enumerate((bstats0, bstats1)):
                    mv = stat.tile([128, 2], F32, name=f"mv{t}", tag="mv")
                    nc.vector.bn_aggr(out=mv, in_=bstats)
                    si = stat.tile([128, 2], F32, name=f"si{t}", tag=f"si{t}", bufs=1)
                    nc.vector.tensor_copy(out=si[:, 0:1], in_=mv[:, 0:1])
                    nc.vector.tensor_tensor(out=si[:, 1:2], in0=mv[:, 0:1], in1=mv[:, 0:1], op=OP.mult)
                    nc.vector.tensor_tensor(out=si[:, 1:2], in0=si[:, 1:2], in1=mv[:, 1:2], op=OP.add)
                    statsin.append(si)

                gsum_ps = ps_small.tile([32, 2], F32, name="gsum_ps", tag="small")
                nc.tensor.matmul(gsum_ps, gind[:, 0, :], statsin[0], start=True, stop=False)
                nc.tensor.matmul(gsum_ps, gind[:, 1, :], statsin[1], start=False, stop=True)
                warm_burst(24)

                grp = stat.tile([32, 2], F32, name="grp", bufs=1)
                nc.vector.tensor_copy(out=grp, in_=gsum_ps)
                var_g = stat.tile([32, 1], F32, name="var_g", bufs=1)
                nc.vector.scalar_tensor_tensor(out=var_g, in0=grp[:, 0:1],
                                               scalar=grp[:, 0:1], in1=grp[:, 1:2],
                                               op0=OP.mult, op1=OP.subtract)
                nc.scalar.activation(out=var_g, in_=var_g, func=AF.Sqrt, bias=epst, scale=-1.0)
                # dummy exp reading the sqrt's output: forces queue order
                # sqrt -> exp so the ACT table swaps to exp_and_others NOW
                # (ACT idle) — identity lives in that set too, so no further
                # table load before or during attention
                nc.scalar.activation(out=escr[0:32, :], in_=var_g, func=AF.Exp, bias=epst)
                nc.vector.reciprocal(out=grp[:, 1:2], in_=var_g)  # grp = (mu_g, rstd_g)

                for t, gt in enumerate((gindT0, gindT1)):
                    bc_ps = ps_small.tile([128, 2], F32, name=f"bc_ps{t}", tag="small")
                    nc.tensor.matmul(bc_ps, gt, grp, start=True, stop=True)
                    warm_burst(10)
                    nc.vector.tensor_tensor(out=scale_c[:, t:t + 1], in0=gsc[:, t:t + 1],
                                            in1=bc_ps[:, 1:2], op=OP.mult)
                    nc.vector.tensor_tensor(out=gnb_c[:, t:t + 1], in0=bc_ps[:, 0:1],
                                            in1=scale_c[:, t:t + 1], op=OP.mult)
                    nc.vector.tensor_tensor(out=gnb_c[:, t:t + 1], in0=gbi[:, t:t + 1],
                                            in1=gnb_c[:, t:t + 1], op=OP.subtract)

                # ---- phase 2: fold GN scale into qkv weights; k/q weights
                # go to fp8 with a x16 prescale (entries ~1/16-sigma would
                # otherwise sit in e4m3's subnormal range); the bias-apply
                # compensates with scale=1/16
                for cchunk in range(2):
                    nc.vector.tensor_scalar(out=wks[:, cchunk, :], in0=wk[:, cchunk, :],
                                            scalar1=scale_c[:, cchunk:cchunk + 1],
                                            scalar2=16.0, op0=OP.mult, op1=OP.mult)
                    nc.vector.tensor_scalar(out=wqs[:, cchunk, :], in0=wq[:, cchunk, :],
                                            scalar1=scale_c[:, cchunk:cchunk + 1],
                                            scalar2=16.0, op0=OP.mult, op1=OP.mult)
                    nc.vector.tensor_scalar(out=wvs[:, cchunk, :], in0=wv[:, cchunk, :],
                                            scalar1=scale_c[:, cchunk:cchunk + 1],
                                            scalar2=16.0, op0=OP.mult, op1=OP.mult)
                nc.gpsimd.tensor_copy(out=ph, in_=pt)  # fp8 proj weights

                def bias_chains():
                    # beta_W = W^T @ gnb (+ input bias); tiny matmuls — they
                    # are emitted INSIDE the projection stream so the big
                    # matmuls (which need only the folded weights) don't
                    # queue behind this PE<->DVE ping-pong
                    for wt, bsrc, bdst in ((wk, bkv, bk_t), (wq, bqv, bq_t), (wv, None, bv_t)):
                        for ot in range(2):
                            b_ps = ps_small.tile([128, 1], F32, name=f"b_ps{ot}", tag="small")
                            nc.tensor.matmul(b_ps, wt[:, 0, ot * 128:(ot + 1) * 128],
                                             gnb_c[:, 0:1], start=True, stop=False)
                            nc.tensor.matmul(b_ps, wt[:, 1, ot * 128:(ot + 1) * 128],
                                             gnb_c[:, 1:2], start=False, stop=True)
                            if bsrc is not None:
                                nc.vector.tensor_tensor(out=bdst[:, ot:ot + 1], in0=b_ps,
                                                        in1=bsrc[:, ot:ot + 1], op=OP.add)
                            else:
                                nc.vector.tensor_copy(out=bdst[:, ot:ot + 1], in_=b_ps)
                    for ot in range(2):
                        d_ps = ps_small.tile([128, 1], F32, name=f"d_ps{ot}", tag="small")
                        nc.tensor.matmul(d_ps, pt[:, 0, ot * 128:(ot + 1) * 128],
                                         bv_t[:, 0:1], start=True, stop=False)
                        nc.tensor.matmul(d_ps, pt[:, 1, ot * 128:(ot + 1) * 128],
                                         bv_t[:, 1:2], start=False, stop=True)
                        nc.vector.tensor_tensor(out=ob_t[:, ot:ot + 1], in0=d_ps,
                                                in1=pbe[:, ot:ot + 1], op=OP.add)

                # ---- phase 3: q / k / vT projections (fp32r in, fp8 out)
                with tc.tile_pool(name="ps_proj3", bufs=4, space="PSUM") as ps3:
                    # blocks whose fp8 outputs attention consumes EARLY get
                    # their bias-apply on ACT (they precede the exps in the
                    # ACT FIFO); late-consumed blocks apply on DVE so the
                    # exp stream starts as soon as k0-3 + q-jb0 are ready
                    big = []  # (dst, bias, weights, ot, jb, on_act)
                    for ot in range(2):
                        for jb in range(N // QB):
                            big.append((kh, bk_t[:, ot:ot + 1], wks, ot, jb, True))
                    for ot in range(2):
                        for jb in range(NJB):
                            big.append((qh, bq_t[:, ot:ot + 1], wqs, ot, jb, True))

                    def apply_bias(dst, bias, ot, sl, p_b, on_act):
                        # undo the x16 fp8 weight prescale here
                        if on_act:
                            nc.scalar.activation(out=dst[:, ot, sl], in_=p_b,
                                                 func=AF.Identity, bias=bias, scale=0.0625)
                        else:
                            nc.vector.tensor_scalar(out=dst[:, ot, sl], in0=p_b,
                                                    scalar1=0.0625, scalar2=bias,
                                                    op0=OP.mult, op1=OP.add)

                    deferred = []
                    for nt in range(KT):
                        if big and nt % 4 != 3:   # 24 big blocks over 32 nt slots
                            dst, bias, wgt, ot, jb, on_act = big.pop(0)
                            sl = slice(jb * QB, (jb + 1) * QB)
                            p_b = ps3.tile([128, QB], F32, name="p_b", tag="pp")
                            nc.tensor.matmul(p_b, wgt[:, :, ot * 128:(ot + 1) * 128],
                                             x8[:, :, sl], start=True, stop=True,
                                             perf_mode=DR)
                            if nt < 2:
                                # bias producers (bias_chains) are emitted at
                                # nt==1 — defer these applies until after
                                deferred.append((dst, bias, ot, sl, p_b, on_act))
                            else:
                                apply_bias(dst, bias, ot, sl, p_b, on_act)
                        nsl = slice(nt * 128, (nt + 1) * 128)
                        p_v = ps3.tile([128, 256], F32, name="p_v", tag="pp")
                        nc.tensor.matmul(p_v, x8[:, :, nsl], wvs, start=True, stop=True,
                                         perf_mode=DR)
                        nc.vector.tensor_scalar_mul(out=vh[:, nt // 2, nt % 2, :], in0=p_v,
                                                    scalar1=0.0625)
                        if nt == 1:
                            bias_chains()
                            for args in deferred:
                                apply_bias(*args)
                            deferred = []
                    assert not big

        # ---- phase 4: attention (fp8 DoubleRow) -------------------------
        with (
            tc.tile_pool(name="ps_s", bufs=2, space="PSUM") as ps_s,
            tc.tile_pool(name="ps_av", bufs=3, space="PSUM") as ps_av,
            tc.tile_pool(name="ps_po", bufs=1, space="PSUM") as ps_po,
            tc.tile_pool(name="e_pool", bufs=3) as e_pool,
            tc.tile_pool(name="an_pool", bufs=2) as an_pool,
            tc.tile_pool(name="o_pool", bufs=4) as o_pool,
            tc.tile_pool(name="rs_pool", bufs=2) as rs_pool,
        ):
            def epilogue(jb, an):
                # proj DR matmuls + bias/residual + store for query block jb.
                qsl = slice(jb * QB, (jb + 1) * QB)
                for ot, xres in enumerate((x0, x1)):
                    po = ps_po.tile([128, QB], F32, name="po", tag="po")
                    nc.tensor.matmul(po, ph[:, :, ot * 128:(ot + 1) * 128],
                                     an, start=True, stop=True, perf_mode=DR)
                    o_sb = o_pool.tile([128, QB], F32, name="o_sb", tag="o_sb")
                    nc.vector.scalar_tensor_tensor(out=o_sb, in0=po,
                                                   scalar=ob_t[:, ot:ot + 1],
                                                   in1=xres[:, qsl],
                                                   op0=OP.add, op1=OP.add)
                    nc.sync.dma_start(out=out_d[ot * 128:(ot + 1) * 128, qsl], in_=o_sb)

            pending = None
            for jb in range(NJB):
                qsl = slice(jb * QB, (jb + 1) * QB)
                av_a = ps_av.tile([128, QB], F32, name="av_a", tag="av")
                av_b = ps_av.tile([128, QB], F32, name="av_b", tag="av")
                rs = ps_av.tile([128, QB], F32, name="rs", tag="av")
                e8s = {}

                def av_group(pr):
                    e8 = e8s.pop(pr)
                    # rowsum: ones-weights DR matmul gives the softmax sums
                    # replicated across all 128 partitions (no broadcast op)
                    nc.tensor.matmul(rs, ones8, e8,
                                     start=(pr == 0), stop=(pr == NPAIR - 1), perf_mode=DR)
                    nc.tensor.matmul(av_a, vh[:, pr, :, 0:128], e8,
                                     start=(pr == 0), stop=(pr == NPAIR - 1), perf_mode=DR)
                    nc.tensor.matmul(av_b, vh[:, pr, :, 128:256], e8,
                                     start=(pr == 0), stop=(pr == NPAIR - 1), perf_mode=DR)

                def emit_s(pr):
                    s_ps = ps_s.tile([128, 2, QB], F32, name="s_ps", tag="s")
                    for i in (0, 1):
                        kt = 2 * pr + i
                        ksl = slice(kt * 128, (kt + 1) * 128)
                        nc.tensor.matmul(s_ps[:, i, :], kh[:, :, ksl], qh[:, :, qsl],
                                         start=True, stop=True, perf_mode=DR)
                    return s_ps

                # scores run one pair AHEAD of the exp stream (and ahead of
                # the av batch in the PE queue) so ACT never waits on them
                s_cur = emit_s(0)
                for pr in range(NPAIR):
                    e8 = e_pool.tile([128, 2, QB], F8, name="e8", tag="e8")
                    # one exp covers the whole pair (2 PSUM banks read as one AP)
                    nc.scalar.activation(out=e8, in_=s_cur, func=AF.Exp, bias=eoff)
                    e8s[pr] = e8
                    if pr + 1 < NPAIR:
                        s_cur = emit_s(pr + 1)
                    if pr >= 1:
                        av_group(pr - 1)
                    if pr == 2 and pending is not None:
                        epilogue(*pending)
                        pending = None
                av_group(NPAIR - 1)

                if jb < NJB - 1:
                    # normalize: a = av * (1/rowsum); rs already has the sums
                    # replicated across partitions (ones-weights DR matmul)
                    rsr = rs_pool.tile([128, QB], F32, name="rsr", tag="rsr")
                    nc.vector.reciprocal_approx_fast(out=rsr, in_=rs)
                    an = an_pool.tile([128, 2, QB], F8, name="an", tag="an")
                    nc.vector.tensor_tensor(out=an[:, 0, :], in0=av_a, in1=rsr, op=OP.mult)
                    nc.vector.tensor_tensor(out=an[:, 1, :], in0=av_b, in1=rsr, op=OP.mult)
                    pending = (jb, an)
                else:
                    # final block: pipeline the normalize/proj/store chain in
                    # two half-width pieces (DVE of half 2 overlaps PE of half 1)
                    HB = QB // 2
                    for h in range(2):
                        hsl = slice(h * HB, (h + 1) * HB)
                        qsl_h = slice(jb * QB + h * HB, jb * QB + (h + 1) * HB)
                        rsr_h = rs_pool.tile([128, HB], F32, name=f"rsrh{h}", tag=f"rsrh{h}", bufs=1)
                        nc.vector.reciprocal_approx_fast(out=rsr_h, in_=rs[:, hsl])
                        an_h = an_pool.tile([128, 2, HB], F8, name=f"an_h{h}", tag="an")
                        nc.vector.tensor_tensor(out=an_h[:, 0, :], in0=av_a[:, hsl], in1=rsr_h, op=OP.mult)
                        nc.vector.tensor_tensor(out=an_h[:, 1, :], in0=av_b[:, hsl], in1=rsr_h, op=OP.mult)
                        for ot, xres in enumerate((x0, x1)):
                            po = ps_po.tile([128, HB], F32, name="po_h", tag="po")
                            nc.tensor.matmul(po, ph[:, :, ot * 128:(ot + 1) * 128],
                                             an_h, start=True, stop=True, perf_mode=DR)
                            o_sb = o_pool.tile([128, HB], F32, name="o_sb_h", tag="o_sb")
                            nc.vector.scalar_tensor_tensor(out=o_sb, in0=po,
                                                           scalar=ob_t[:, ot:ot + 1],
                                                           in1=xres[:, qsl_h],
                                                           op0=OP.add, op1=OP.add)
                            nc.sync.dma_start(out=out_d[ot * 128:(ot + 1) * 128, qsl_h],
                                              in_=o_sb)
            assert pending is None


_CACHED_NC = None


def _build_program():
    global _CACHED_NC
    if _CACHED_NC is not None:
        return _CACHED_NC
    nc = bacc.Bacc("TRN2", target_bir_lowering=False, debug=False,
                   num_devices=NCORES)
    d = {
        "x": nc.dram_tensor("x", [C, N], F32R, kind="ExternalInput").ap(),
        "wqT": nc.dram_tensor("wqT", [C, C], F32, kind="ExternalInput").ap(),
        "wkT": nc.dram_tensor("wkT", [C, C], F32, kind="ExternalInput").ap(),
        "wvT": nc.dram_tensor("wvT", [C, C], F32, kind="ExternalInput").ap(),
        "pT": nc.dram_tensor("pT", [C, C], F32, kind="ExternalInput").ap(),
        "vecs": nc.dram_tensor("vecs", [128, 10], F32, kind="ExternalInput").ap(),
        "gind": nc.dram_tensor("gind", [128, 64], F32, kind="ExternalInput").ap(),
        "gindT": nc.dram_tensor("gindT", [2 * 32, 128], F32, kind="ExternalInput").ap(),
        "out": nc.dram_tensor("out", [C, NQ], F32, kind="ExternalOutput").ap(),
    }
    with tile.TileContext(nc) as tc:
        _emit(nc, tc, d)
    nc.compile()
    _CACHED_NC = nc
    return nc


def _prep_host(x, gn_scale, gn_bias, qkv_w, qkv_b, proj_w, proj_b):
    """Host-side weight prep + per-core input maps."""
    f = np.float32
    x = np.asarray(x, f).reshape(B, C, N)
    qkv_w = np.asarray(qkv_w, f)
    qkv_b = np.asarray(qkv_b, f)
    proj_w = np.asarray(proj_w, f)
    proj_b = np.asarray(proj_b, f)
    # split the 1/sqrt(C) score scale evenly between q and k so both sit in a
    # good fp8e4m3 range
    half_scale = np.float32(C ** -0.25)

    Wq = qkv_w[0::3] * half_scale
    bq = qkv_b[0::3] * half_scale
    Wk = qkv_w[1::3] * half_scale
    bk = qkv_b[1::3] * half_scale
    Wv, bv = qkv_w[2::3], qkv_b[2::3]

    wqT = np.ascontiguousarray(Wq.T, f)
    wkT = np.ascontiguousarray(Wk.T, f)
    wvT = np.ascontiguousarray(Wv.T, f)
    pT = np.ascontiguousarray(proj_w.T, f)
    pbe = (proj_b + proj_w @ bv).astype(f)
    vstack = np.stack([np.asarray(gn_scale, f), np.asarray(gn_bias, f),
                       bq.astype(f), bk.astype(f), pbe], axis=0)  # [5, 256]
    vecs = np.ascontiguousarray(
        vstack.reshape(5, 2, 128).transpose(2, 0, 1).reshape(128, 10))
    gind, gindT = _indicator_constants()

    shared = {"wqT": wqT, "wkT": wkT, "wvT": wvT, "pT": pT, "vecs": vecs,
              "gind": gind, "gindT": gindT}
    in_maps = []
    for ci in range(NCORES):
        b, half = divmod(ci, 2)
        xb = x[b]
        if half == 1:
            xb = np.concatenate([xb[:, NQ:], xb[:, :NQ]], axis=1)
        in_maps.append({"x": np.ascontiguousarray(xb), **shared})
    return in_maps


def _assemble(results):
    out = np.empty((B, C, N), np.float32)
    for ci in range(NCORES):
        b, half = divmod(ci, 2)
        out[b][:, half * NQ:(half + 1) * NQ] = results[ci]["out"]
    return out.reshape(B, C, H, W)


def kernel(x, gn_scale, gn_bias, qkv_w, qkv_b, proj_w, proj_b):
    nc = _build_program()
    in_maps = _prep_host(x, gn_scale, gn_bias, qkv_w, qkv_b, proj_w, proj_b)
    res = run_bass_kernel_spmd(nc, in_maps, core_ids=list(range(NCORES)))
    return _assemble(res.results)


if __name__ == "__main__":
    rng = np.random.default_rng(0)
    inputs = {
        "x": rng.standard_normal((B, C, H, W), dtype=np.float32),
        "gn_scale": np.ones(C, np.float32),
        "gn_bias": np.zeros(C, np.float32),
        "qkv_w": rng.standard_normal((3 * C, C), dtype=np.float32) * C ** -0.5,
        "qkv_b": np.zeros(3 * C, np.float32),
        "proj_w": rng.standard_normal((C, C), dtype=np.float32) * C ** -0.5,
        "proj_b": np.zeros(C, np.float32),
    }
    out = kernel(**inputs)
    print("out", out.shape, out.dtype, float(np.abs(out).mean()))
